# revision 1
# baseline (speedup 1.0000x reference)
"""BiLSTM (2-layer, H=64, T=1024, B=512) TRN2 Bass kernel.

Data-parallel over batch across 8 NeuronCores (B_shard=64/core); LSTM/FC
weights replicated. Per core, three phases:
  A: layer-1 fwd+bwd as one merged scan (PSUM banks = gate types, bank rows =
     [fwd-stream; bwd-stream]); input projections (gx) + biases enter PSUM via
     bulk matmuls (8 steps per bank), per-step recurrent matmuls (block-diag
     lhsT) accumulate on top. One Sigmoid covers all 4 gate banks (g-gate
     weights pre-scaled x2; tanh(g) = 2*sigma(2g)-1 fixed up on DVE).
  B: layer-2 fwd, same structure with bank rows = [batch 0:32; batch 32:64].
  C: layer-2 bwd needs only its t=T-1 step; FC head on device.
The bwd stream's time reversal is done with negative-stride DMA APs.
"""

import sys
import numpy as np

sys.path.insert(0, "/opt/trn_rl_repo")

import concourse.bass as bass  # noqa: E402
import concourse.mybir as mybir  # noqa: E402
from concourse import bacc  # noqa: E402
from concourse.tile import TileContext  # noqa: E402
from concourse.bass_utils import run_bass_kernel_spmd  # noqa: E402

F32 = mybir.dt.float32
F32R = mybir.dt.float32r
BF16 = mybir.dt.bfloat16
AF = mybir.ActivationFunctionType
MUL = mybir.AluOpType.mult
ADD = mybir.AluOpType.add

T, IN, H, G = 1024, 128, 64, 256
B_FULL = 512
N_CORES = 8
BSH = B_FULL // N_CORES   # 64
CH = 8                    # timesteps per PSUM bank
NB = CH * BSH             # 512
HB = BSH // 2             # 32
NB2 = CH * HB             # 256
GX_DT = "f32r"            # bulk input-projection matmul mode
REC_DT = "f32"            # recurrent matmul mode


def _build(gx_dt=GX_DT, rec_dt=REC_DT, num_devices=N_CORES):
    NCH = T // CH
    nc = bacc.Bacc("TRN2", target_bir_lowering=False, debug=False,
                   num_devices=num_devices)

    x_d = nc.dram_tensor("x", [T, IN, BSH], F32, kind="ExternalInput").ap()
    w1_ih_d = nc.dram_tensor("w1_ih", [IN, 2, 4, 128], F32, kind="ExternalInput").ap()
    w1_hh_d = nc.dram_tensor("w1_hh", [128, 4, 128], F32, kind="ExternalInput").ap()
    w2_ih_d = nc.dram_tensor("w2_ih", [128, 2, 4, 128], F32, kind="ExternalInput").ap()
    w2_hh_d = nc.dram_tensor("w2_hh", [128, 4, 128], F32, kind="ExternalInput").ap()
    w2b_ih_d = nc.dram_tensor("w2b_ih", [128, 2, 4, 128], F32, kind="ExternalInput").ap()
    bias_d = nc.dram_tensor("bias_rows", [1, 12, 128], F32, kind="ExternalInput").ap()
    fcb_d = nc.dram_tensor("fc_b", [BSH, 1], F32, kind="ExternalInput").ap()
    fc_w_d = nc.dram_tensor("fc_w", [128, 1], F32, kind="ExternalInput").ap()
    out_d = nc.dram_tensor("out", [BSH, 1], F32, kind="ExternalOutput").ap()

    def gxc(ap):
        return ap

    W_DT = BF16 if rec_dt == "bf16" else F32

    def rev_ap(base_ap, t_hi, p0, p1, ch):
        tstr = 128 * BSH
        return bass.AP(
            tensor=base_ap.tensor,
            offset=base_ap.offset + t_hi * tstr + p0 * BSH,
            ap=[[BSH, p1 - p0], [-tstr, ch], [1, BSH]])

    with TileContext(nc) as tc:
        with tc.tile_pool(name="singles", bufs=1) as singles, \
             tc.tile_pool(name="dram", bufs=1, space="DRAM") as drampool:

            h1_d = drampool.tile([T, 128, BSH], F32)

            w1_ih = singles.tile([IN, 2, 4, 128], F32)
            w1_hh = singles.tile([128, 4, 128], W_DT)
            w2_ih = singles.tile([128, 2, 4, 128], F32)
            w2_hh = singles.tile([128, 4, 128], W_DT)
            w2b_ih = singles.tile([128, 2, 4, 128], F32)
            bias_r = singles.tile([1, 12, 128], F32)
            bias_rb = singles.tile([1, 12, 128], BF16)
            ones = singles.tile([1, NB], BF16)
            fc_w = singles.tile([128, 1], F32)
            fc_b = singles.tile([BSH, 1], F32)

            nc.sync.dma_start(out=w1_ih, in_=w1_ih_d)
            nc.sync.dma_start(out=w2_ih, in_=w2_ih_d)
            nc.sync.dma_start(out=w2b_ih, in_=w2b_ih_d)
            if GX_DT == "f32r":
                w1_ih_r = singles.tile([IN, 2, 4, 128], F32R)
                w2_ih_r = singles.tile([128, 2, 4, 128], F32R)
                w2b_ih_r = singles.tile([128, 2, 4, 128], F32R)
                nc.vector.tensor_copy(w1_ih_r, w1_ih)
                nc.vector.tensor_copy(w2_ih_r, w2_ih)
                nc.vector.tensor_copy(w2b_ih_r, w2b_ih)
                w1_ih, w2_ih, w2b_ih = w1_ih_r, w2_ih_r, w2b_ih_r
            if rec_dt == "bf16":
                w1_hh_f = singles.tile([128, 4, 128], F32)
                w2_hh_f = singles.tile([128, 4, 128], F32)
                nc.sync.dma_start(out=w1_hh_f, in_=w1_hh_d)
                nc.sync.dma_start(out=w2_hh_f, in_=w2_hh_d)
                nc.vector.tensor_copy(w1_hh, w1_hh_f)
                nc.vector.tensor_copy(w2_hh, w2_hh_f)
            else:
                nc.sync.dma_start(out=w1_hh, in_=w1_hh_d)
                nc.sync.dma_start(out=w2_hh, in_=w2_hh_d)
            nc.sync.dma_start(out=bias_r, in_=bias_d)
            nc.vector.tensor_copy(bias_rb, bias_r)
            nc.sync.dma_start(out=fc_b, in_=fcb_d)
            nc.sync.dma_start(out=fc_w, in_=fc_w_d)
            nc.vector.memset(ones, 1.0)

            h2cat = singles.tile([128, BSH], F32)

            # =============== PHASE A ===============
            with tc.tile_pool(name="xa", bufs=3) as xpool, \
                 tc.tile_pool(name="ga", bufs=2, space="PSUM") as gpsum, \
                 tc.tile_pool(name="acta", bufs=3) as apool, \
                 tc.tile_pool(name="sta", bufs=4) as spool:

                hst_prev = spool.tile([128, CH, BSH], F32, tag="hst", name="hst0")
                nc.vector.memset(hst_prev, 0.0)
                m_t = spool.tile([128, 2, BSH], F32, tag="m", name="m_init")
                nc.vector.memset(m_t, 0.0)

                for c in range(NCH):
                    t0 = c * CH
                    xf = xpool.tile([IN, CH, BSH], F32, tag="xf")
                    xb = xpool.tile([IN, CH, BSH], F32, tag="xb")
                    nc.sync.dma_start(
                        out=xf, in_=x_d[t0:t0 + CH].rearrange("t p b -> p t b"))
                    nc.sync.dma_start(out=xb,
                                      in_=rev_ap(x_d, T - 1 - t0, 0, IN, CH))
                    if gx_dt == "f32r":
                        xfr = xpool.tile([IN, CH, BSH], F32R, tag="xfr")
                        xbr = xpool.tile([IN, CH, BSH], F32R, tag="xbr")
                        nc.vector.tensor_copy(xfr, xf)
                        nc.vector.tensor_copy(xbr, xb)
                    else:
                        xfr, xbr = xf, xb
                    xf2 = xfr.rearrange("p t b -> p (t b)")
                    xb2 = xbr.rearrange("p t b -> p (t b)")

                    pall = gpsum.tile([128, 4, NB], F32, tag="pall")
                    for g in range(4):
                        nc.tensor.matmul(pall[:, g], bias_rb[:, g],
                                         ones, start=True, stop=True)
                        nc.tensor.matmul(pall[:, g], w1_ih[:, 0, g], xf2,
                                         start=False, stop=False,
                                         skip_group_check=True)
                        nc.tensor.matmul(pall[:, g], w1_ih[:, 1, g], xb2,
                                         start=False, stop=False,
                                         skip_group_check=True)

                    hst = spool.tile([128, CH, BSH], F32, tag="hst")
                    pview = pall.rearrange("p g (t b) -> p g t b", t=CH)

                    for s in range(CH):
                        h_prev = hst_prev[:, CH - 1] if s == 0 else hst[:, s - 1]
                        for g in range(4):
                            nc.tensor.matmul(pview[:, g, s], w1_hh[:, g],
                                             h_prev, start=False, stop=False,
                                             skip_group_check=True)

                        a_all = apool.tile([128, 4, BSH], F32, tag="a_all")
                        nc.scalar.activation(a_all, pview[:, :, s], AF.Sigmoid)

                        m_n = spool.tile([128, 2, BSH], F32, tag="m", name="m_n")
                        nc.vector.tensor_scalar(out=m_t[:, 0], in0=a_all[:, 2],
                                                scalar1=2.0, scalar2=-1.0,
                                                op0=MUL, op1=ADD)
                        up = apool.tile([128, 2, BSH], F32, tag="up")
                        nc.vector.tensor_tensor(out=up, in0=a_all[:, 0:2],
                                                in1=m_t, op=MUL)
                        nc.vector.tensor_add(m_n[:, 1], up[:, 0], up[:, 1])
                        tc_t = apool.tile([128, BSH], F32, tag="tc_t")
                        nc.scalar.activation(tc_t, m_n[:, 1], AF.Tanh)
                        nc.vector.tensor_mul(hst[:, s], a_all[:, 3], tc_t)
                        m_t = m_n

                    nc.sync.dma_start(
                        out=h1_d[t0:t0 + CH, 0:64].rearrange("t p b -> p t b"),
                        in_=hst[0:64])
                    nc.sync.dma_start(
                        out=rev_ap(h1_d, T - 1 - t0, 64, 128, CH),
                        in_=hst[64:128])
                    hst_prev = hst

            # =============== PHASE B ===============
            with tc.tile_pool(name="hb", bufs=3) as hpool, \
                 tc.tile_pool(name="gb", bufs=2, space="PSUM") as gpsum2, \
                 tc.tile_pool(name="actb", bufs=3) as apool2, \
                 tc.tile_pool(name="stb", bufs=4) as spool2:

                h2_prev = spool2.tile([128, HB], F32, tag="h2", name="h2_init")
                nc.vector.memset(h2_prev, 0.0)
                m2_t = spool2.tile([128, 2, HB], F32, tag="m2", name="m2_init")
                nc.vector.memset(m2_t, 0.0)

                for c in range(NCH):
                    t0 = c * CH
                    h1c = hpool.tile([128, CH, BSH], F32, tag="h1c")
                    nc.sync.dma_start(
                        out=h1c, in_=h1_d[t0:t0 + CH].rearrange("t p b -> p t b"))
                    if gx_dt == "f32r":
                        h1cr = hpool.tile([128, CH, BSH], F32R, tag="h1cr")
                        nc.vector.tensor_copy(h1cr, h1c)
                    else:
                        h1cr = h1c
                    r0 = h1cr[:, :, 0:HB]
                    r1 = h1cr[:, :, HB:BSH]

                    p2 = gpsum2.tile([128, 4, NB], F32, tag="p2")
                    for g in range(4):
                        nc.tensor.matmul(p2[:, g, 0:NB2], bias_rb[:, 4 + g],
                                         ones[:, 0:NB2], start=True,
                                         stop=True)
                        nc.tensor.matmul(p2[:, g, 0:NB2], w2_ih[:, 0, g], r0,
                                         start=False, stop=False,
                                         skip_group_check=True)
                        nc.tensor.matmul(p2[:, g, 0:NB2], w2_ih[:, 1, g], r1,
                                         start=False, stop=False,
                                         skip_group_check=True)

                    p2v = p2.rearrange("p g (t b) -> p g t b", t=2 * CH)

                    for s in range(CH):
                        for g in range(4):
                            nc.tensor.matmul(p2v[:, g, s], w2_hh[:, g],
                                             h2_prev, start=False, stop=False,
                                             skip_group_check=True)

                        a2 = apool2.tile([128, 4, HB], F32, tag="a2")
                        nc.scalar.activation(a2, p2v[:, :, s], AF.Sigmoid)
                        m2_n = spool2.tile([128, 2, HB], F32, tag="m2",
                                           name="m2_n")
                        nc.vector.tensor_scalar(out=m2_t[:, 0], in0=a2[:, 2],
                                                scalar1=2.0, scalar2=-1.0,
                                                op0=MUL, op1=ADD)
                        up2 = apool2.tile([128, 2, HB], F32, tag="up2")
                        nc.vector.tensor_tensor(out=up2, in0=a2[:, 0:2],
                                                in1=m2_t, op=MUL)
                        nc.vector.tensor_add(m2_n[:, 1], up2[:, 0], up2[:, 1])
                        tc2 = apool2.tile([128, HB], F32, tag="tc2")
                        nc.scalar.activation(tc2, m2_n[:, 1], AF.Tanh)
                        h2_n = spool2.tile([128, HB], F32, tag="h2", name="h2_n")
                        nc.vector.tensor_mul(h2_n, a2[:, 3], tc2)
                        h2_prev = h2_n
                        m2_t = m2_n

                # =============== PHASE C ===============
                h1l = apool2.tile([128, BSH], F32)
                nc.sync.dma_start(out=h1l, in_=h1_d[T - 1])
                if gx_dt == "f32r":
                    h1lr = apool2.tile([128, BSH], F32R)
                    nc.vector.tensor_copy(h1lr, h1l)
                else:
                    h1lr = h1l
                p3 = gpsum2.tile([128, 4, NB], F32, tag="p2")
                for g in range(4):
                    nc.tensor.matmul(p3[:, g, 0:HB], bias_rb[:, 8 + g],
                                     ones[:, 0:HB], start=True, stop=True)
                    nc.tensor.matmul(p3[:, g, 0:HB], w2b_ih[:, 0, g],
                                     h1lr[:, 0:HB], start=False, stop=False,
                                     skip_group_check=True)
                    nc.tensor.matmul(p3[:, g, 0:HB], w2b_ih[:, 1, g],
                                     h1lr[:, HB:BSH], start=False,
                                     stop=False, skip_group_check=True)
                a3 = apool2.tile([128, 4, HB], F32)
                nc.scalar.activation(a3, p3[:, :, 0:HB], AF.Sigmoid)
                g3 = apool2.tile([128, HB], F32)
                nc.vector.tensor_scalar(out=g3, in0=a3[:, 2], scalar1=2.0,
                                        scalar2=-1.0, op0=MUL, op1=ADD)
                c3 = apool2.tile([128, HB], F32)
                nc.vector.tensor_mul(c3, a3[:, 0], g3)
                t3 = apool2.tile([128, HB], F32)
                nc.scalar.activation(t3, c3, AF.Tanh)
                h2b = apool2.tile([128, HB], F32)
                nc.vector.tensor_mul(h2b, a3[:, 3], t3)

                nc.sync.dma_start(out=h2cat[0:64, 0:HB], in_=h2_prev[0:64])
                nc.sync.dma_start(out=h2cat[0:64, HB:BSH], in_=h2_prev[64:128])
                nc.sync.dma_start(out=h2cat[64:128, 0:HB], in_=h2b[0:64])
                nc.sync.dma_start(out=h2cat[64:128, HB:BSH], in_=h2b[64:128])

                out_ps = gpsum2.tile([BSH, 1], F32, tag="p2")
                nc.tensor.matmul(out_ps, h2cat, fc_w, start=True, stop=True)
                out_sb = apool2.tile([BSH, 1], F32)
                nc.scalar.activation(out_sb, out_ps, AF.Identity, bias=fc_b)
                nc.sync.dma_start(out=out_d, in_=out_sb)

    nc.finalize()
    return nc


def _x2(wT):
    w = np.ascontiguousarray(wT).astype(np.float32).copy()
    w[..., 128:192] *= 2.0
    return w


def _blkdiag(wfT, wbT):
    out = np.zeros((128, 4, 128), np.float32)
    for g in range(4):
        out[0:64, g, 0:64] = wfT[:, g * 64:(g + 1) * 64]
        out[64:128, g, 64:128] = wbT[:, g * 64:(g + 1) * 64]
    return out


def _prep_shared(w_ih, w_hh, b_ih, b_hh, fc_w, fc_b):
    b = (np.asarray(b_ih) + np.asarray(b_hh)).astype(np.float32)
    w_ih = np.asarray(w_ih, np.float32)
    w_hh = np.asarray(w_hh, np.float32)

    def _padih(wT_a, wT_b, K):
        # [K, 2, 4, 128]: stream a -> cols 0:64, stream b -> cols 64:128
        out = np.zeros((K, 2, 4, 128), np.float32)
        for g in range(4):
            out[:, 0, g, 0:64] = wT_a[:, g * 64:(g + 1) * 64]
            out[:, 1, g, 64:128] = wT_b[:, g * 64:(g + 1) * 64]
        return out

    w1 = _padih(_x2(w_ih[0, 0].T), _x2(w_ih[0, 1].T), IN)
    w1h = _blkdiag(_x2(w_hh[0, 0].T), _x2(w_hh[0, 1].T))
    w2T = _x2(w_ih[1, 0].T)
    w2 = _padih(w2T, w2T, 128)
    w2hT = _x2(w_hh[1, 0].T)
    w2h = _blkdiag(w2hT, w2hT)
    w2bT = _x2(w_ih[1, 1].T)
    w2b = _padih(w2bT, w2bT, 128)

    def bias_rows(bvec_f, bvec_b):
        out = np.zeros((4, 128), np.float32)
        for g in range(4):
            sc = 2.0 if g == 2 else 1.0
            out[g, 0:64] = sc * bvec_f[g * 64:(g + 1) * 64]
            out[g, 64:128] = sc * bvec_b[g * 64:(g + 1) * 64]
        return out

    br = np.zeros((1, 12, 128), np.float32)
    br[0, 0:4] = bias_rows(b[0, 0], b[0, 1])
    br[0, 4:8] = bias_rows(b[1, 0], b[1, 0])
    br[0, 8:12] = bias_rows(b[1, 1], b[1, 1])
    return {
        "w1_ih": np.ascontiguousarray(w1),
        "w1_hh": np.ascontiguousarray(w1h),
        "w2_ih": np.ascontiguousarray(w2),
        "w2_hh": np.ascontiguousarray(w2h),
        "w2b_ih": np.ascontiguousarray(w2b),
        "bias_rows": br,
        "fc_b": np.full((BSH, 1), float(np.asarray(fc_b).ravel()[0]), np.float32),
        "fc_w": np.ascontiguousarray(np.asarray(fc_w, np.float32).T),
    }


_NC_CACHE = {}


def _get_nc():
    key = (GX_DT, REC_DT)
    if key not in _NC_CACHE:
        _NC_CACHE[key] = _build(gx_dt=GX_DT, rec_dt=REC_DT)
    return _NC_CACHE[key]


def _run(inputs, trace=False, tmpdir=None):
    x = np.asarray(inputs["x"], np.float32)
    shared = _prep_shared(inputs["w_ih"], inputs["w_hh"], inputs["b_ih"],
                          inputs["b_hh"], inputs["fc_w"], inputs["fc_b"])
    in_maps = []
    for c in range(N_CORES):
        xs = np.ascontiguousarray(
            x[c * BSH:(c + 1) * BSH].transpose(1, 2, 0))  # [T, IN, BSH]
        m = dict(shared)
        m["x"] = xs
        in_maps.append(m)
    nc = _get_nc()
    res = run_bass_kernel_spmd(nc, in_maps, list(range(N_CORES)),
                               trace=trace, tmpdir=tmpdir)
    out = np.concatenate([res.results[c]["out"] for c in range(N_CORES)],
                         axis=0).astype(np.float32)
    return out, res


def kernel(x, w_ih, w_hh, b_ih, b_hh, fc_w, fc_b):
    out, _ = _run({"x": x, "w_ih": w_ih, "w_hh": w_hh, "b_ih": b_ih,
                   "b_hh": b_hh, "fc_w": fc_w, "fc_b": fc_b})
    return out



# revision 8
# speedup vs baseline: 1.6760x; 1.6760x over previous
"""BiLSTM (2-layer, H=64, T=1024, B=512) TRN2 Bass kernel.

Data-parallel over batch across 8 NeuronCores (B_shard=64/core); LSTM/FC
weights replicated. Per core, three phases:
  A: layer-1 fwd+bwd as one merged scan (PSUM banks = gate types in order
     [g,i,f,o], bank rows = [fwd-stream; bwd-stream]); input projections +
     biases enter PSUM via bulk matmuls (8 steps per bank), per-step
     recurrent matmuls (block-diag lhsT) accumulate on top.
  B: layer-2 fwd, same structure with bank rows = [batch 0:32; batch 32:64].
  C: layer-2 bwd needs only its t=T-1 step; FC head on device.

All matmul operands are bf16 (single-pass PE + fast weight load); PSUM and
the cell state stay fp32. Layer-1 hidden states live entirely in SBUF
(h1_sb, bf16) — no DRAM round-trip; the bwd stream's time reversal is
handled with negative-stride APs at consumption time.

Cell math per step (state c' = c/2, g-gate weights pre-scaled x2):
  sigmoid over [2g, i] and [f, o] banks -> a (bf16)
  u  = (sigma(2g) - 0.5) * i          (one scalar_tensor_tensor; = i*tanh(g)/2)
  v  = f * c'                          (tensor_tensor)
  c' = u + v                           (tensor_tensor)
  t  = Tanh(2*c')                      (activation with scale=2)
  h  = o * t  -> written into h1_sb[pos]
"""

import sys
import numpy as np

sys.path.insert(0, "/opt/trn_rl_repo")

import ml_dtypes  # noqa: E402

import concourse.bass as bass  # noqa: E402
import concourse.mybir as mybir  # noqa: E402
from concourse import bacc  # noqa: E402
from concourse.tile import TileContext  # noqa: E402
from concourse.bass_utils import run_bass_kernel_spmd  # noqa: E402

F32 = mybir.dt.float32
BF16 = mybir.dt.bfloat16
AF = mybir.ActivationFunctionType
MUL = mybir.AluOpType.mult
ADD = mybir.AluOpType.add

T, IN, H, G = 1024, 128, 64, 256
B_FULL = 512
N_CORES = 8
BSH = B_FULL // N_CORES   # 64
CH = 8                    # timesteps per PSUM bank
NB = CH * BSH             # 512
HB = BSH // 2             # 32
NB2 = CH * HB             # 256
NCH = T // CH             # 128
BF = np.dtype(ml_dtypes.bfloat16)


def _build(num_devices=N_CORES):
    nc = bacc.Bacc("TRN2", target_bir_lowering=False, debug=False,
                   num_devices=num_devices)

    x_d = nc.dram_tensor("x", [T, IN, BSH], BF16, kind="ExternalInput").ap()
    w1_ih_d = nc.dram_tensor("w1_ih", [IN, 2, 4, 128], BF16, kind="ExternalInput").ap()
    w1_hh_d = nc.dram_tensor("w1_hh", [128, 4, 128], BF16, kind="ExternalInput").ap()
    w2_ih_d = nc.dram_tensor("w2_ih", [128, 2, 4, 128], BF16, kind="ExternalInput").ap()
    w2_hh_d = nc.dram_tensor("w2_hh", [128, 4, 128], BF16, kind="ExternalInput").ap()
    w2b_ih_d = nc.dram_tensor("w2b_ih", [128, 2, 4, 128], BF16, kind="ExternalInput").ap()
    bias_d = nc.dram_tensor("bias_rows", [1, 12, 128], BF16, kind="ExternalInput").ap()
    fcb_d = nc.dram_tensor("fc_b", [BSH, 1], F32, kind="ExternalInput").ap()
    fc_w_d = nc.dram_tensor("fc_w", [128, 1], BF16, kind="ExternalInput").ap()
    out_d = nc.dram_tensor("out", [BSH, 1], F32, kind="ExternalOutput").ap()

    def rev_x_ap(t_hi, ch):
        # x_d[t_hi - k, :, :] for k in 0..ch-1, laid out [IN, ch, BSH]
        tstr = IN * BSH
        return bass.AP(
            tensor=x_d.tensor,
            offset=x_d.offset + t_hi * tstr,
            ap=[[BSH, IN], [-tstr, ch], [1, BSH]])

    with TileContext(nc) as tc:
        with tc.tile_pool(name="singles", bufs=1) as singles:

            h1_sb = singles.tile([128, T, BSH], BF16)   # 128 KiB/partition

            w1_ih = singles.tile([IN, 2, 4, 128], BF16)
            w1_hh = singles.tile([128, 4, 128], BF16)
            w2_ih = singles.tile([128, 2, 4, 128], BF16)
            w2_hh = singles.tile([128, 4, 128], BF16)
            w2b_ih = singles.tile([128, 2, 4, 128], BF16)
            bias_r = singles.tile([1, 12, 128], BF16)
            ones = singles.tile([1, NB], BF16)
            fc_w = singles.tile([128, 1], BF16)
            fc_b = singles.tile([BSH, 1], F32)

            nc.sync.dma_start(out=w1_ih, in_=w1_ih_d)
            nc.sync.dma_start(out=w1_hh, in_=w1_hh_d)
            nc.sync.dma_start(out=w2_ih, in_=w2_ih_d)
            nc.sync.dma_start(out=w2_hh, in_=w2_hh_d)
            nc.sync.dma_start(out=w2b_ih, in_=w2b_ih_d)
            nc.sync.dma_start(out=bias_r, in_=bias_d)
            nc.sync.dma_start(out=fc_b, in_=fcb_d)
            nc.sync.dma_start(out=fc_w, in_=fc_w_d)
            nc.vector.memset(ones, 1.0)

            h2cat = singles.tile([128, BSH], BF16)

            # =============== PHASE A ===============
            with tc.tile_pool(name="xa", bufs=3) as xpool, \
                 tc.tile_pool(name="ga", bufs=2, space="PSUM") as gpsum, \
                 tc.tile_pool(name="acta", bufs=3) as apool, \
                 tc.tile_pool(name="sta", bufs=4) as spool:

                c_t = spool.tile([128, BSH], F32, tag="c", name="c_init")
                nc.vector.memset(c_t, 0.0)

                for c in range(NCH):
                    t0 = c * CH
                    xf = xpool.tile([IN, CH, BSH], BF16, tag="xf")
                    xb = xpool.tile([IN, CH, BSH], BF16, tag="xb")
                    nc.sync.dma_start(
                        out=xf, in_=x_d[t0:t0 + CH].rearrange("t p b -> p t b"))
                    nc.sync.dma_start(out=xb, in_=rev_x_ap(T - 1 - t0, CH))
                    xf2 = xf.rearrange("p t b -> p (t b)")
                    xb2 = xb.rearrange("p t b -> p (t b)")

                    pall = gpsum.tile([128, 4, NB], F32, tag="pall")
                    for g in range(4):
                        nc.tensor.matmul(pall[:, g], bias_r[:, g],
                                         ones, start=True, stop=True)
                        nc.tensor.matmul(pall[:, g], w1_ih[:, 0, g], xf2,
                                         start=False, stop=False,
                                         skip_group_check=True)
                        nc.tensor.matmul(pall[:, g], w1_ih[:, 1, g], xb2,
                                         start=False, stop=False,
                                         skip_group_check=True)

                    pview = pall.rearrange("p g (t b) -> p g t b", t=CH)

                    for s in range(CH):
                        pos = t0 + s
                        if pos > 0:
                            for g in range(4):
                                nc.tensor.matmul(pview[:, g, s], w1_hh[:, g],
                                                 h1_sb[:, pos - 1],
                                                 start=False, stop=False,
                                                 skip_group_check=True)

                        a_all = apool.tile([128, 4, BSH], BF16, tag="a_all")
                        nc.scalar.activation(a_all[:, 0:2], pview[:, 0:2, s],
                                             AF.Sigmoid)
                        nc.scalar.activation(a_all[:, 2:4], pview[:, 2:4, s],
                                             AF.Sigmoid)

                        u = apool.tile([128, BSH], BF16, tag="u")
                        nc.vector.scalar_tensor_tensor(
                            out=u, in0=a_all[:, 0], scalar=-0.5,
                            in1=a_all[:, 1], op0=ADD, op1=MUL)
                        v = apool.tile([128, BSH], F32, tag="v")
                        nc.vector.tensor_tensor(out=v, in0=a_all[:, 2],
                                                in1=c_t, op=MUL)
                        c_n = spool.tile([128, BSH], F32, tag="c", name="c_n")
                        nc.vector.tensor_tensor(out=c_n, in0=u, in1=v, op=ADD)
                        tc_t = apool.tile([128, BSH], BF16, tag="tc_t")
                        nc.scalar.activation(tc_t, c_n, AF.Tanh, scale=2.0)
                        nc.vector.tensor_tensor(out=h1_sb[:, pos],
                                                in0=a_all[:, 3], in1=tc_t,
                                                op=MUL)
                        c_t = c_n

            # =============== PHASE B ===============
            with tc.tile_pool(name="hb", bufs=3) as hpool, \
                 tc.tile_pool(name="gb", bufs=2, space="PSUM") as gpsum2, \
                 tc.tile_pool(name="actb", bufs=3) as apool2, \
                 tc.tile_pool(name="stb", bufs=4) as spool2:

                h2_prev = spool2.tile([128, HB], BF16, tag="h2", name="h2_init")
                nc.vector.memset(h2_prev, 0.0)
                c2_t = spool2.tile([128, HB], F32, tag="c2", name="c2_init")
                nc.vector.memset(c2_t, 0.0)

                # h1_sb SBUF AP space: partition stride = free size (T*BSH),
                # free layout [T, BSH] with t-stride BSH.
                ppitch = T * BSH

                for c in range(NCH):
                    t0 = c * CH
                    h1c = hpool.tile([128, CH, BSH], BF16, tag="h1c")
                    nc.vector.tensor_copy(h1c[0:64],
                                          h1_sb[0:64, t0:t0 + CH])
                    rev_in = bass.AP(
                        tensor=h1_sb.tensor,
                        offset=h1_sb.offset + 64 * ppitch
                        + (T - 1 - t0) * BSH,
                        ap=[[ppitch, 64], [-BSH, CH], [1, BSH]])
                    nc.vector.tensor_copy(h1c[64:128], rev_in)

                    r0 = h1c[:, :, 0:HB]
                    r1 = h1c[:, :, HB:BSH]

                    p2 = gpsum2.tile([128, 4, NB], F32, tag="p2")
                    for g in range(4):
                        nc.tensor.matmul(p2[:, g, 0:NB2], bias_r[:, 4 + g],
                                         ones[:, 0:NB2], start=True,
                                         stop=True)
                        nc.tensor.matmul(p2[:, g, 0:NB2], w2_ih[:, 0, g], r0,
                                         start=False, stop=False,
                                         skip_group_check=True)
                        nc.tensor.matmul(p2[:, g, 0:NB2], w2_ih[:, 1, g], r1,
                                         start=False, stop=False,
                                         skip_group_check=True)

                    p2v = p2.rearrange("p g (t b) -> p g t b", t=2 * CH)

                    for s in range(CH):
                        if c > 0 or s > 0:
                            for g in range(4):
                                nc.tensor.matmul(p2v[:, g, s], w2_hh[:, g],
                                                 h2_prev, start=False,
                                                 stop=False,
                                                 skip_group_check=True)

                        a2 = apool2.tile([128, 4, HB], BF16, tag="a2")
                        nc.scalar.activation(a2[:, 0:2], p2v[:, 0:2, s],
                                             AF.Sigmoid)
                        nc.scalar.activation(a2[:, 2:4], p2v[:, 2:4, s],
                                             AF.Sigmoid)

                        u2 = apool2.tile([128, HB], BF16, tag="u2")
                        nc.vector.scalar_tensor_tensor(
                            out=u2, in0=a2[:, 0], scalar=-0.5,
                            in1=a2[:, 1], op0=ADD, op1=MUL)
                        v2 = apool2.tile([128, HB], F32, tag="v2")
                        nc.vector.tensor_tensor(out=v2, in0=a2[:, 2],
                                                in1=c2_t, op=MUL)
                        c2_n = spool2.tile([128, HB], F32, tag="c2",
                                           name="c2_n")
                        nc.vector.tensor_tensor(out=c2_n, in0=u2, in1=v2,
                                                op=ADD)
                        tc2 = apool2.tile([128, HB], BF16, tag="tc2")
                        nc.scalar.activation(tc2, c2_n, AF.Tanh, scale=2.0)
                        h2_n = spool2.tile([128, HB], BF16, tag="h2",
                                           name="h2_n")
                        nc.vector.tensor_tensor(out=h2_n, in0=a2[:, 3],
                                                in1=tc2, op=MUL)
                        h2_prev = h2_n
                        c2_t = c2_n

                # =============== PHASE C ===============
                h1l = apool2.tile([128, BSH], BF16)
                nc.vector.tensor_copy(h1l[0:64], h1_sb[0:64, T - 1])
                nc.vector.tensor_copy(h1l[64:128], h1_sb[64:128, 0])

                p3 = gpsum2.tile([128, 4, NB], F32, tag="p2")
                for g in range(4):
                    nc.tensor.matmul(p3[:, g, 0:HB], bias_r[:, 8 + g],
                                     ones[:, 0:HB], start=True, stop=True)
                    nc.tensor.matmul(p3[:, g, 0:HB], w2b_ih[:, 0, g],
                                     h1l[:, 0:HB], start=False, stop=False,
                                     skip_group_check=True)
                    nc.tensor.matmul(p3[:, g, 0:HB], w2b_ih[:, 1, g],
                                     h1l[:, HB:BSH], start=False,
                                     stop=False, skip_group_check=True)
                a3 = apool2.tile([128, 4, HB], BF16)
                nc.scalar.activation(a3, p3[:, :, 0:HB], AF.Sigmoid)
                u3 = apool2.tile([128, HB], F32)
                nc.vector.scalar_tensor_tensor(
                    out=u3, in0=a3[:, 0], scalar=-0.5, in1=a3[:, 1],
                    op0=ADD, op1=MUL)
                t3 = apool2.tile([128, HB], BF16)
                nc.scalar.activation(t3, u3, AF.Tanh, scale=2.0)
                h2b = apool2.tile([128, HB], BF16)
                nc.vector.tensor_tensor(out=h2b, in0=a3[:, 3], in1=t3, op=MUL)

                nc.sync.dma_start(out=h2cat[0:64, 0:HB], in_=h2_prev[0:64])
                nc.sync.dma_start(out=h2cat[0:64, HB:BSH], in_=h2_prev[64:128])
                nc.sync.dma_start(out=h2cat[64:128, 0:HB], in_=h2b[0:64])
                nc.sync.dma_start(out=h2cat[64:128, HB:BSH], in_=h2b[64:128])

                out_ps = gpsum2.tile([BSH, 1], F32, tag="p2")
                nc.tensor.matmul(out_ps, h2cat, fc_w, start=True, stop=True)
                out_sb = apool2.tile([BSH, 1], F32)
                nc.scalar.activation(out_sb, out_ps, AF.Identity, bias=fc_b)
                nc.sync.dma_start(out=out_d, in_=out_sb)

    nc.finalize()
    return nc


# PSUM gate-bank order [g, i, f, o]; PyTorch rows are [i, f, g, o].
GATE_SRC = [2, 0, 1, 3]
GATE_SCALE = [2.0, 1.0, 1.0, 1.0]  # g pre-scaled for the sigmoid/tanh trick


def _padih(wT_a, wT_b, K):
    # [K, 2, 4, 128]: stream a -> cols 0:64, stream b -> cols 64:128
    out = np.zeros((K, 2, 4, 128), np.float32)
    for k in range(4):
        gs, sc = GATE_SRC[k], GATE_SCALE[k]
        out[:, 0, k, 0:64] = sc * wT_a[:, gs * 64:(gs + 1) * 64]
        out[:, 1, k, 64:128] = sc * wT_b[:, gs * 64:(gs + 1) * 64]
    return out


def _blkdiag(wfT, wbT):
    out = np.zeros((128, 4, 128), np.float32)
    for k in range(4):
        gs, sc = GATE_SRC[k], GATE_SCALE[k]
        out[0:64, k, 0:64] = sc * wfT[:, gs * 64:(gs + 1) * 64]
        out[64:128, k, 64:128] = sc * wbT[:, gs * 64:(gs + 1) * 64]
    return out


def _prep_shared(w_ih, w_hh, b_ih, b_hh, fc_w, fc_b):
    b = (np.asarray(b_ih) + np.asarray(b_hh)).astype(np.float32)
    w_ih = np.asarray(w_ih, np.float32)
    w_hh = np.asarray(w_hh, np.float32)

    w1 = _padih(w_ih[0, 0].T, w_ih[0, 1].T, IN)
    w1h = _blkdiag(w_hh[0, 0].T, w_hh[0, 1].T)
    w2T = w_ih[1, 0].T
    w2 = _padih(w2T, w2T, 128)
    w2hT = w_hh[1, 0].T
    w2h = _blkdiag(w2hT, w2hT)
    w2bT = w_ih[1, 1].T
    w2b = _padih(w2bT, w2bT, 128)

    def bias_rows(bvec_f, bvec_b):
        out = np.zeros((4, 128), np.float32)
        for k in range(4):
            gs, sc = GATE_SRC[k], GATE_SCALE[k]
            out[k, 0:64] = sc * bvec_f[gs * 64:(gs + 1) * 64]
            out[k, 64:128] = sc * bvec_b[gs * 64:(gs + 1) * 64]
        return out

    br = np.zeros((1, 12, 128), np.float32)
    br[0, 0:4] = bias_rows(b[0, 0], b[0, 1])
    br[0, 4:8] = bias_rows(b[1, 0], b[1, 0])
    br[0, 8:12] = bias_rows(b[1, 1], b[1, 1])
    return {
        "w1_ih": np.ascontiguousarray(w1).astype(BF),
        "w1_hh": np.ascontiguousarray(w1h).astype(BF),
        "w2_ih": np.ascontiguousarray(w2).astype(BF),
        "w2_hh": np.ascontiguousarray(w2h).astype(BF),
        "w2b_ih": np.ascontiguousarray(w2b).astype(BF),
        "bias_rows": br.astype(BF),
        "fc_b": np.full((BSH, 1), float(np.asarray(fc_b).ravel()[0]), np.float32),
        "fc_w": np.ascontiguousarray(np.asarray(fc_w, np.float32).T).astype(BF),
    }


_NC_CACHE = {}


def _get_nc():
    key = "v2"
    if key not in _NC_CACHE:
        _NC_CACHE[key] = _build()
    return _NC_CACHE[key]


def _run(inputs, trace=False, tmpdir=None):
    x = np.asarray(inputs["x"], np.float32)
    shared = _prep_shared(inputs["w_ih"], inputs["w_hh"], inputs["b_ih"],
                          inputs["b_hh"], inputs["fc_w"], inputs["fc_b"])
    in_maps = []
    for c in range(N_CORES):
        xs = np.ascontiguousarray(
            x[c * BSH:(c + 1) * BSH].transpose(1, 2, 0)).astype(BF)  # [T, IN, BSH]
        m = dict(shared)
        m["x"] = xs
        in_maps.append(m)
    nc = _get_nc()
    res = run_bass_kernel_spmd(nc, in_maps, list(range(N_CORES)),
                               trace=trace, tmpdir=tmpdir)
    out = np.concatenate([res.results[c]["out"] for c in range(N_CORES)],
                         axis=0).astype(np.float32)
    return out, res


def kernel(x, w_ih, w_hh, b_ih, b_hh, fc_w, fc_b):
    out, _ = _run({"x": x, "w_ih": w_ih, "w_hh": w_hh, "b_ih": b_ih,
                   "b_hh": b_hh, "fc_w": fc_w, "fc_b": fc_b})
    return out


# revision 11
# speedup vs baseline: 1.6778x; 1.0011x over previous
"""BiLSTM (2-layer, H=64, T=1024, B=512) TRN2 Bass kernel.

Data-parallel over batch across 8 NeuronCores (B_shard=64/core); LSTM/FC
weights replicated. Per core, three phases:
  A: layer-1 fwd+bwd as one merged scan (PSUM banks = gate types in order
     [g,i,f,o], bank rows = [fwd-stream; bwd-stream]); input projections +
     biases enter PSUM via bulk matmuls (8 steps per bank), per-step
     recurrent matmuls (block-diag lhsT) accumulate on top.
  B: layer-2 fwd, same structure with bank rows = [batch 0:32; batch 32:64].
  C: layer-2 bwd needs only its t=T-1 step; FC head on device.

All matmul operands are bf16 (single-pass PE + fast weight load); PSUM and
the cell state stay fp32. Layer-1 hidden states live entirely in SBUF
(h1_sb, bf16) — no DRAM round-trip; the bwd stream's time reversal is
handled with negative-stride APs at consumption time.

Cell math per step (state c' = c/2, g-gate weights pre-scaled x2):
  sigmoid over [2g, i] and [f, o] banks -> a (bf16)
  u  = (sigma(2g) - 0.5) * i          (one scalar_tensor_tensor; = i*tanh(g)/2)
  v  = f * c'                          (tensor_tensor)
  c' = u + v                           (tensor_tensor)
  t  = Tanh(2*c')                      (activation with scale=2)
  h  = o * t  -> written into h1_sb[pos]
"""

import sys
import numpy as np

sys.path.insert(0, "/opt/trn_rl_repo")

import ml_dtypes  # noqa: E402

import concourse.bass as bass  # noqa: E402
import concourse.mybir as mybir  # noqa: E402
from concourse import bacc  # noqa: E402
from concourse.tile import TileContext  # noqa: E402
from concourse.bass_utils import run_bass_kernel_spmd  # noqa: E402

F32 = mybir.dt.float32
BF16 = mybir.dt.bfloat16
AF = mybir.ActivationFunctionType
MUL = mybir.AluOpType.mult
ADD = mybir.AluOpType.add

T, IN, H, G = 1024, 128, 64, 256
B_FULL = 512
N_CORES = 8
BSH = B_FULL // N_CORES   # 64
CH = 8                    # timesteps per PSUM bank
NB = CH * BSH             # 512
HB = BSH // 2             # 32
NB2 = CH * HB             # 256
NCH = T // CH             # 128
BF = np.dtype(ml_dtypes.bfloat16)


def _build(num_devices=N_CORES):
    nc = bacc.Bacc("TRN2", target_bir_lowering=False, debug=False,
                   num_devices=num_devices)

    x_d = nc.dram_tensor("x", [T, IN, BSH], BF16, kind="ExternalInput").ap()
    w1_ih_d = nc.dram_tensor("w1_ih", [IN, 2, 4, 128], BF16, kind="ExternalInput").ap()
    w1_hh_d = nc.dram_tensor("w1_hh", [128, 4, 128], BF16, kind="ExternalInput").ap()
    w2_ih_d = nc.dram_tensor("w2_ih", [128, 2, 4, 128], BF16, kind="ExternalInput").ap()
    w2_hh_d = nc.dram_tensor("w2_hh", [128, 4, 128], BF16, kind="ExternalInput").ap()
    w2b_ih_d = nc.dram_tensor("w2b_ih", [128, 2, 4, 128], BF16, kind="ExternalInput").ap()
    bias_d = nc.dram_tensor("bias_rows", [1, 12, 128], BF16, kind="ExternalInput").ap()
    fcb_d = nc.dram_tensor("fc_b", [BSH, 1], F32, kind="ExternalInput").ap()
    fc_w_d = nc.dram_tensor("fc_w", [128, 1], BF16, kind="ExternalInput").ap()
    out_d = nc.dram_tensor("out", [BSH, 1], F32, kind="ExternalOutput").ap()

    def rev_x_ap(t_hi, ch):
        # x_d[t_hi - k, :, :] for k in 0..ch-1, laid out [IN, ch, BSH]
        tstr = IN * BSH
        return bass.AP(
            tensor=x_d.tensor,
            offset=x_d.offset + t_hi * tstr,
            ap=[[BSH, IN], [-tstr, ch], [1, BSH]])

    with TileContext(nc) as tc:
        with tc.tile_pool(name="singles", bufs=1) as singles:

            h1_sb = singles.tile([128, T, BSH], BF16)   # 128 KiB/partition

            w1_ih = singles.tile([IN, 2, 4, 128], BF16)
            w1_hh = singles.tile([128, 4, 128], BF16)
            w2_ih = singles.tile([128, 2, 4, 128], BF16)
            w2_hh = singles.tile([128, 4, 128], BF16)
            w2b_ih = singles.tile([128, 2, 4, 128], BF16)
            bias_r = singles.tile([1, 12, 128], BF16)
            ones = singles.tile([1, NB], BF16)
            fc_w = singles.tile([128, 1], BF16)
            fc_b = singles.tile([BSH, 1], F32)

            nc.sync.dma_start(out=w1_ih, in_=w1_ih_d)
            nc.sync.dma_start(out=w1_hh, in_=w1_hh_d)
            nc.sync.dma_start(out=w2_ih, in_=w2_ih_d)
            nc.sync.dma_start(out=w2_hh, in_=w2_hh_d)
            nc.sync.dma_start(out=w2b_ih, in_=w2b_ih_d)
            nc.sync.dma_start(out=bias_r, in_=bias_d)
            nc.sync.dma_start(out=fc_b, in_=fcb_d)
            nc.sync.dma_start(out=fc_w, in_=fc_w_d)
            nc.vector.memset(ones, 1.0)

            h2cat = singles.tile([128, BSH], BF16)

            # PE warmup: ~10us of dense matmuls flips the HAM clock gate to
            # 8/8 (2.4 GHz); steady-state PE gaps stay under the ~3.4us MID
            # window so it never re-throttles.
            warm_sb = singles.tile([128, NB], BF16)
            nc.vector.memset(warm_sb, 0.0)
            with tc.tile_pool(name="warm", bufs=1, space="PSUM") as wpsum:
                wp = wpsum.tile([128, NB], F32)
                for _ in range(24):
                    nc.tensor.matmul(wp, warm_sb[:, 0:128], warm_sb,
                                     start=True, stop=True)

            # =============== PHASE A ===============
            with tc.tile_pool(name="xa", bufs=3) as xpool, \
                 tc.tile_pool(name="ga", bufs=2, space="PSUM") as gpsum, \
                 tc.tile_pool(name="acta", bufs=3) as apool, \
                 tc.tile_pool(name="sta", bufs=4) as spool:

                c_t = spool.tile([128, BSH], F32, tag="c", name="c_init")
                nc.vector.memset(c_t, 0.0)

                def load_x(c):
                    t0 = c * CH
                    xf = xpool.tile([IN, CH, BSH], BF16, tag="xf")
                    xb = xpool.tile([IN, CH, BSH], BF16, tag="xb")
                    nc.sync.dma_start(
                        out=xf, in_=x_d[t0:t0 + CH].rearrange("t p b -> p t b"))
                    nc.sync.dma_start(out=xb, in_=rev_x_ap(T - 1 - t0, CH))
                    return xf, xb

                def bulk_ops_A(xf, xb, pall):
                    xf2 = xf.rearrange("p t b -> p (t b)")
                    xb2 = xb.rearrange("p t b -> p (t b)")
                    ops = []
                    for g in range(4):
                        ops.append(lambda g=g: nc.tensor.matmul(
                            pall[:, g], bias_r[:, g], ones,
                            start=True, stop=True))
                        ops.append(lambda g=g: nc.tensor.matmul(
                            pall[:, g], w1_ih[:, 0, g], xf2,
                            start=False, stop=False, skip_group_check=True))
                        ops.append(lambda g=g: nc.tensor.matmul(
                            pall[:, g], w1_ih[:, 1, g], xb2,
                            start=False, stop=False, skip_group_check=True))
                    return ops

                xf, xb = load_x(0)
                pall = gpsum.tile([128, 4, NB], F32, tag="pall", name="p0")
                for op in bulk_ops_A(xf, xb, pall):
                    op()

                for c in range(NCH):
                    t0 = c * CH
                    pview = pall.rearrange("p g (t b) -> p g t b", t=CH)
                    if c + 1 < NCH:
                        xf, xb = load_x(c + 1)
                        pall_n = gpsum.tile([128, 4, NB], F32, tag="pall",
                                            name=f"p{c + 1}")
                        pending = bulk_ops_A(xf, xb, pall_n)
                    else:
                        pall_n, pending = None, []

                    for s in range(CH):
                        pos = t0 + s
                        if pos > 0:
                            for g in range(4):
                                nc.tensor.matmul(pview[:, g, s], w1_hh[:, g],
                                                 h1_sb[:, pos - 1],
                                                 start=False, stop=False,
                                                 skip_group_check=True)
                        # interleave next chunk's input-projection matmuls
                        q0, q1 = (s * 12) // CH, ((s + 1) * 12) // CH
                        for op in pending[q0:q1]:
                            op()

                        a_all = apool.tile([128, 4, BSH], BF16, tag="a_all")
                        nc.scalar.activation(a_all[:, 0:2], pview[:, 0:2, s],
                                             AF.Sigmoid)
                        nc.scalar.activation(a_all[:, 2:4], pview[:, 2:4, s],
                                             AF.Sigmoid)

                        u = apool.tile([128, BSH], BF16, tag="u")
                        nc.vector.scalar_tensor_tensor(
                            out=u, in0=a_all[:, 0], scalar=-0.5,
                            in1=a_all[:, 1], op0=ADD, op1=MUL)
                        v = apool.tile([128, BSH], F32, tag="v")
                        nc.vector.tensor_tensor(out=v, in0=a_all[:, 2],
                                                in1=c_t, op=MUL)
                        c_n = spool.tile([128, BSH], F32, tag="c", name="c_n")
                        nc.vector.tensor_tensor(out=c_n, in0=u, in1=v, op=ADD)
                        tc_t = apool.tile([128, BSH], BF16, tag="tc_t")
                        nc.scalar.activation(tc_t, c_n, AF.Tanh, scale=2.0)
                        nc.vector.tensor_tensor(out=h1_sb[:, pos],
                                                in0=a_all[:, 3], in1=tc_t,
                                                op=MUL)
                        c_t = c_n
                    pall = pall_n

            # =============== PHASE B ===============
            with tc.tile_pool(name="hb", bufs=3) as hpool, \
                 tc.tile_pool(name="gb", bufs=2, space="PSUM") as gpsum2, \
                 tc.tile_pool(name="actb", bufs=3) as apool2, \
                 tc.tile_pool(name="stb", bufs=4) as spool2:

                h2_prev = spool2.tile([128, HB], BF16, tag="h2", name="h2_init")
                nc.vector.memset(h2_prev, 0.0)
                c2_t = spool2.tile([128, HB], F32, tag="c2", name="c2_init")
                nc.vector.memset(c2_t, 0.0)

                # h1_sb SBUF AP space: partition stride = free size (T*BSH),
                # free layout [T, BSH] with t-stride BSH.
                ppitch = T * BSH

                def load_h1c(c):
                    t0 = c * CH
                    h1c = hpool.tile([128, CH, BSH], BF16, tag="h1c")
                    nc.vector.tensor_copy(h1c[0:64],
                                          h1_sb[0:64, t0:t0 + CH])
                    rev_in = bass.AP(
                        tensor=h1_sb.tensor,
                        offset=h1_sb.offset + 64 * ppitch
                        + (T - 1 - t0) * BSH,
                        ap=[[ppitch, 64], [-BSH, CH], [1, BSH]])
                    nc.vector.tensor_copy(h1c[64:128], rev_in)
                    return h1c

                def bulk_ops_B(h1c, p2):
                    r0 = h1c[:, :, 0:HB]
                    r1 = h1c[:, :, HB:BSH]
                    ops = []
                    for g in range(4):
                        ops.append(lambda g=g: nc.tensor.matmul(
                            p2[:, g, 0:NB2], bias_r[:, 4 + g], ones[:, 0:NB2],
                            start=True, stop=True))
                        ops.append(lambda g=g: nc.tensor.matmul(
                            p2[:, g, 0:NB2], w2_ih[:, 0, g], r0,
                            start=False, stop=False, skip_group_check=True))
                        ops.append(lambda g=g: nc.tensor.matmul(
                            p2[:, g, 0:NB2], w2_ih[:, 1, g], r1,
                            start=False, stop=False, skip_group_check=True))
                    return ops

                h1c = load_h1c(0)
                p2 = gpsum2.tile([128, 4, NB], F32, tag="p2", name="q0")
                for op in bulk_ops_B(h1c, p2):
                    op()

                for c in range(NCH):
                    p2v = p2.rearrange("p g (t b) -> p g t b", t=2 * CH)
                    if c + 1 < NCH:
                        h1c = load_h1c(c + 1)
                        p2_n = gpsum2.tile([128, 4, NB], F32, tag="p2",
                                           name=f"q{c + 1}")
                        pending = bulk_ops_B(h1c, p2_n)
                    else:
                        p2_n, pending = None, []

                    for s in range(CH):
                        if c > 0 or s > 0:
                            for g in range(4):
                                nc.tensor.matmul(p2v[:, g, s], w2_hh[:, g],
                                                 h2_prev, start=False,
                                                 stop=False,
                                                 skip_group_check=True)
                        q0, q1 = (s * 12) // CH, ((s + 1) * 12) // CH
                        for op in pending[q0:q1]:
                            op()

                        a2 = apool2.tile([128, 4, HB], BF16, tag="a2")
                        nc.scalar.activation(a2[:, 0:2], p2v[:, 0:2, s],
                                             AF.Sigmoid)
                        nc.scalar.activation(a2[:, 2:4], p2v[:, 2:4, s],
                                             AF.Sigmoid)

                        u2 = apool2.tile([128, HB], BF16, tag="u2")
                        nc.vector.scalar_tensor_tensor(
                            out=u2, in0=a2[:, 0], scalar=-0.5,
                            in1=a2[:, 1], op0=ADD, op1=MUL)
                        v2 = apool2.tile([128, HB], F32, tag="v2")
                        nc.vector.tensor_tensor(out=v2, in0=a2[:, 2],
                                                in1=c2_t, op=MUL)
                        c2_n = spool2.tile([128, HB], F32, tag="c2",
                                           name="c2_n")
                        nc.vector.tensor_tensor(out=c2_n, in0=u2, in1=v2,
                                                op=ADD)
                        tc2 = apool2.tile([128, HB], BF16, tag="tc2")
                        nc.scalar.activation(tc2, c2_n, AF.Tanh, scale=2.0)
                        h2_n = spool2.tile([128, HB], BF16, tag="h2",
                                           name="h2_n")
                        nc.vector.tensor_tensor(out=h2_n, in0=a2[:, 3],
                                                in1=tc2, op=MUL)
                        h2_prev = h2_n
                        c2_t = c2_n
                    p2 = p2_n

                # =============== PHASE C ===============
                h1l = apool2.tile([128, BSH], BF16)
                nc.vector.tensor_copy(h1l[0:64], h1_sb[0:64, T - 1])
                nc.vector.tensor_copy(h1l[64:128], h1_sb[64:128, 0])

                p3 = gpsum2.tile([128, 4, NB], F32, tag="p2")
                for g in range(4):
                    nc.tensor.matmul(p3[:, g, 0:HB], bias_r[:, 8 + g],
                                     ones[:, 0:HB], start=True, stop=True)
                    nc.tensor.matmul(p3[:, g, 0:HB], w2b_ih[:, 0, g],
                                     h1l[:, 0:HB], start=False, stop=False,
                                     skip_group_check=True)
                    nc.tensor.matmul(p3[:, g, 0:HB], w2b_ih[:, 1, g],
                                     h1l[:, HB:BSH], start=False,
                                     stop=False, skip_group_check=True)
                a3 = apool2.tile([128, 4, HB], BF16)
                nc.scalar.activation(a3, p3[:, :, 0:HB], AF.Sigmoid)
                u3 = apool2.tile([128, HB], F32)
                nc.vector.scalar_tensor_tensor(
                    out=u3, in0=a3[:, 0], scalar=-0.5, in1=a3[:, 1],
                    op0=ADD, op1=MUL)
                t3 = apool2.tile([128, HB], BF16)
                nc.scalar.activation(t3, u3, AF.Tanh, scale=2.0)
                h2b = apool2.tile([128, HB], BF16)
                nc.vector.tensor_tensor(out=h2b, in0=a3[:, 3], in1=t3, op=MUL)

                nc.sync.dma_start(out=h2cat[0:64, 0:HB], in_=h2_prev[0:64])
                nc.sync.dma_start(out=h2cat[0:64, HB:BSH], in_=h2_prev[64:128])
                nc.sync.dma_start(out=h2cat[64:128, 0:HB], in_=h2b[0:64])
                nc.sync.dma_start(out=h2cat[64:128, HB:BSH], in_=h2b[64:128])

                out_ps = gpsum2.tile([BSH, 1], F32, tag="p2")
                nc.tensor.matmul(out_ps, h2cat, fc_w, start=True, stop=True)
                out_sb = apool2.tile([BSH, 1], F32)
                nc.scalar.activation(out_sb, out_ps, AF.Identity, bias=fc_b)
                nc.sync.dma_start(out=out_d, in_=out_sb)

    nc.finalize()
    return nc


# PSUM gate-bank order [g, i, f, o]; PyTorch rows are [i, f, g, o].
GATE_SRC = [2, 0, 1, 3]
GATE_SCALE = [2.0, 1.0, 1.0, 1.0]  # g pre-scaled for the sigmoid/tanh trick


def _padih(wT_a, wT_b, K):
    # [K, 2, 4, 128]: stream a -> cols 0:64, stream b -> cols 64:128
    out = np.zeros((K, 2, 4, 128), np.float32)
    for k in range(4):
        gs, sc = GATE_SRC[k], GATE_SCALE[k]
        out[:, 0, k, 0:64] = sc * wT_a[:, gs * 64:(gs + 1) * 64]
        out[:, 1, k, 64:128] = sc * wT_b[:, gs * 64:(gs + 1) * 64]
    return out


def _blkdiag(wfT, wbT):
    out = np.zeros((128, 4, 128), np.float32)
    for k in range(4):
        gs, sc = GATE_SRC[k], GATE_SCALE[k]
        out[0:64, k, 0:64] = sc * wfT[:, gs * 64:(gs + 1) * 64]
        out[64:128, k, 64:128] = sc * wbT[:, gs * 64:(gs + 1) * 64]
    return out


def _prep_shared(w_ih, w_hh, b_ih, b_hh, fc_w, fc_b):
    b = (np.asarray(b_ih) + np.asarray(b_hh)).astype(np.float32)
    w_ih = np.asarray(w_ih, np.float32)
    w_hh = np.asarray(w_hh, np.float32)

    w1 = _padih(w_ih[0, 0].T, w_ih[0, 1].T, IN)
    w1h = _blkdiag(w_hh[0, 0].T, w_hh[0, 1].T)
    w2T = w_ih[1, 0].T
    w2 = _padih(w2T, w2T, 128)
    w2hT = w_hh[1, 0].T
    w2h = _blkdiag(w2hT, w2hT)
    w2bT = w_ih[1, 1].T
    w2b = _padih(w2bT, w2bT, 128)

    def bias_rows(bvec_f, bvec_b):
        out = np.zeros((4, 128), np.float32)
        for k in range(4):
            gs, sc = GATE_SRC[k], GATE_SCALE[k]
            out[k, 0:64] = sc * bvec_f[gs * 64:(gs + 1) * 64]
            out[k, 64:128] = sc * bvec_b[gs * 64:(gs + 1) * 64]
        return out

    br = np.zeros((1, 12, 128), np.float32)
    br[0, 0:4] = bias_rows(b[0, 0], b[0, 1])
    br[0, 4:8] = bias_rows(b[1, 0], b[1, 0])
    br[0, 8:12] = bias_rows(b[1, 1], b[1, 1])
    return {
        "w1_ih": np.ascontiguousarray(w1).astype(BF),
        "w1_hh": np.ascontiguousarray(w1h).astype(BF),
        "w2_ih": np.ascontiguousarray(w2).astype(BF),
        "w2_hh": np.ascontiguousarray(w2h).astype(BF),
        "w2b_ih": np.ascontiguousarray(w2b).astype(BF),
        "bias_rows": br.astype(BF),
        "fc_b": np.full((BSH, 1), float(np.asarray(fc_b).ravel()[0]), np.float32),
        "fc_w": np.ascontiguousarray(np.asarray(fc_w, np.float32).T).astype(BF),
    }


_NC_CACHE = {}


def _get_nc():
    key = "v2"
    if key not in _NC_CACHE:
        _NC_CACHE[key] = _build()
    return _NC_CACHE[key]


def _run(inputs, trace=False, tmpdir=None):
    x = np.asarray(inputs["x"], np.float32)
    shared = _prep_shared(inputs["w_ih"], inputs["w_hh"], inputs["b_ih"],
                          inputs["b_hh"], inputs["fc_w"], inputs["fc_b"])
    in_maps = []
    for c in range(N_CORES):
        xs = np.ascontiguousarray(
            x[c * BSH:(c + 1) * BSH].transpose(1, 2, 0)).astype(BF)  # [T, IN, BSH]
        m = dict(shared)
        m["x"] = xs
        in_maps.append(m)
    nc = _get_nc()
    res = run_bass_kernel_spmd(nc, in_maps, list(range(N_CORES)),
                               trace=trace, tmpdir=tmpdir)
    out = np.concatenate([res.results[c]["out"] for c in range(N_CORES)],
                         axis=0).astype(np.float32)
    return out, res


def kernel(x, w_ih, w_hh, b_ih, b_hh, fc_w, fc_b):
    out, _ = _run({"x": x, "w_ih": w_ih, "w_hh": w_hh, "b_ih": b_ih,
                   "b_hh": b_hh, "fc_w": fc_w, "fc_b": fc_b})
    return out


# revision 14
# speedup vs baseline: 1.7698x; 1.0548x over previous
"""BiLSTM (2-layer, H=64, T=1024, B=512) TRN2 Bass kernel.

Data-parallel over batch across 8 NeuronCores (B_shard=64/core); LSTM/FC
weights replicated. Per core, three phases:
  A: layer-1 fwd+bwd as one merged scan (PSUM banks = gate types in order
     [g,i,f,o], bank rows = [fwd-stream; bwd-stream]); input projections +
     biases enter PSUM via bulk matmuls (8 steps per bank), per-step
     recurrent matmuls (block-diag lhsT) accumulate on top.
  B: layer-2 fwd, same structure with bank rows = [batch 0:32; batch 32:64].
  C: layer-2 bwd needs only its t=T-1 step; FC head on device.

All matmul operands are bf16 (single-pass PE + fast weight load); PSUM and
the cell state stay fp32. Layer-1 hidden states live entirely in SBUF
(h1_sb, bf16) — no DRAM round-trip; the bwd stream's time reversal is
handled with negative-stride APs at consumption time.

Cell math per step (state c' = c/2, g-gate weights pre-scaled x2):
  sigmoid over [2g, i] and [f, o] banks -> a (bf16)
  u  = (sigma(2g) - 0.5) * i          (one scalar_tensor_tensor; = i*tanh(g)/2)
  v  = f * c'                          (tensor_tensor)
  c' = u + v                           (tensor_tensor)
  t  = Tanh(2*c')                      (activation with scale=2)
  h  = o * t  -> written into h1_sb[pos]
"""

import sys
import numpy as np

sys.path.insert(0, "/opt/trn_rl_repo")

import ml_dtypes  # noqa: E402

import concourse.bass as bass  # noqa: E402
import concourse.mybir as mybir  # noqa: E402
from concourse import bacc  # noqa: E402
from concourse.tile import TileContext  # noqa: E402
from concourse.bass_utils import run_bass_kernel_spmd  # noqa: E402

F32 = mybir.dt.float32
BF16 = mybir.dt.bfloat16
AF = mybir.ActivationFunctionType
MUL = mybir.AluOpType.mult
ADD = mybir.AluOpType.add

T, IN, H, G = 1024, 128, 64, 256
B_FULL = 512
N_CORES = 8
BSH = B_FULL // N_CORES   # 64
CH = 8                    # timesteps per PSUM bank
NB = CH * BSH             # 512
HB = BSH // 2             # 32
NB2 = CH * HB             # 256
NCH = T // CH             # 128
BF = np.dtype(ml_dtypes.bfloat16)


def _build(num_devices=N_CORES):
    nc = bacc.Bacc("TRN2", target_bir_lowering=False, debug=False,
                   num_devices=num_devices)

    x_d = nc.dram_tensor("x", [T, IN, BSH], BF16, kind="ExternalInput").ap()
    w1_ih_d = nc.dram_tensor("w1_ih", [IN, 2, 4, 128], BF16, kind="ExternalInput").ap()
    w1_hh_d = nc.dram_tensor("w1_hh", [128, 4, 128], BF16, kind="ExternalInput").ap()
    w2_ih_d = nc.dram_tensor("w2_ih", [128, 2, 4, 128], BF16, kind="ExternalInput").ap()
    w2_hh_d = nc.dram_tensor("w2_hh", [128, 4, 128], BF16, kind="ExternalInput").ap()
    w2b_ih_d = nc.dram_tensor("w2b_ih", [128, 2, 4, 128], BF16, kind="ExternalInput").ap()
    bias_d = nc.dram_tensor("bias_rows", [1, 12, 128], BF16, kind="ExternalInput").ap()
    fcb_d = nc.dram_tensor("fc_b", [BSH, 1], F32, kind="ExternalInput").ap()
    fc_w_d = nc.dram_tensor("fc_w", [128, 1], BF16, kind="ExternalInput").ap()
    out_d = nc.dram_tensor("out", [BSH, 1], F32, kind="ExternalOutput").ap()

    def rev_x_ap(t_hi, ch):
        # x_d[t_hi - k, :, :] for k in 0..ch-1, laid out [IN, ch, BSH]
        tstr = IN * BSH
        return bass.AP(
            tensor=x_d.tensor,
            offset=x_d.offset + t_hi * tstr,
            ap=[[BSH, IN], [-tstr, ch], [1, BSH]])

    with TileContext(nc) as tc:
        with tc.tile_pool(name="singles", bufs=1) as singles:

            h1_sb = singles.tile([128, T, BSH], BF16)   # 128 KiB/partition

            w1_ih = singles.tile([IN, 2, 4, 128], BF16)
            w1_hh = singles.tile([128, 4, 128], BF16)
            w2_ih = singles.tile([128, 2, 4, 128], BF16)
            w2_hh = singles.tile([128, 4, 128], BF16)
            w2b_ih = singles.tile([128, 2, 4, 128], BF16)
            bias_r = singles.tile([1, 12, 128], BF16)
            ones = singles.tile([1, NB], BF16)
            fc_w = singles.tile([128, 1], BF16)
            fc_b = singles.tile([BSH, 1], F32)

            nc.sync.dma_start(out=w1_ih, in_=w1_ih_d)
            nc.sync.dma_start(out=w1_hh, in_=w1_hh_d)
            nc.sync.dma_start(out=w2_ih, in_=w2_ih_d)
            nc.sync.dma_start(out=w2_hh, in_=w2_hh_d)
            nc.sync.dma_start(out=w2b_ih, in_=w2b_ih_d)
            nc.sync.dma_start(out=bias_r, in_=bias_d)
            nc.sync.dma_start(out=fc_b, in_=fcb_d)
            nc.sync.dma_start(out=fc_w, in_=fc_w_d)
            nc.vector.memset(ones, 1.0)

            h2cat = singles.tile([128, BSH], BF16)

            # PE warmup: ~10us of dense matmuls flips the HAM clock gate to
            # 8/8 (2.4 GHz); steady-state PE gaps stay under the ~3.4us MID
            # window so it never re-throttles.
            warm_sb = singles.tile([128, NB], BF16)
            nc.vector.memset(warm_sb, 0.0)
            with tc.tile_pool(name="warm", bufs=1, space="PSUM") as wpsum:
                wp = wpsum.tile([128, NB], F32)
                for _ in range(24):
                    nc.tensor.matmul(wp, warm_sb[:, 0:128], warm_sb,
                                     start=True, stop=True)

            # =============== PHASE A ===============
            with tc.tile_pool(name="xa", bufs=4) as xpool, \
                 tc.tile_pool(name="ga", bufs=2, space="PSUM") as gpsum, \
                 tc.tile_pool(name="acta", bufs=3) as apool, \
                 tc.tile_pool(name="sta", bufs=4) as spool:

                c_t = spool.tile([128, BSH], F32, tag="c", name="c_init")
                nc.vector.memset(c_t, 0.0)

                def load_x(c):
                    t0 = c * CH
                    xf = xpool.tile([IN, CH, BSH], BF16, tag="xf")
                    xb = xpool.tile([IN, CH, BSH], BF16, tag="xb")
                    nc.sync.dma_start(
                        out=xf, in_=x_d[t0:t0 + CH].rearrange("t p b -> p t b"))
                    nc.sync.dma_start(out=xb, in_=rev_x_ap(T - 1 - t0, CH))
                    return xf, xb

                def bulk_ops_A(xf, xb, pall):
                    xf2 = xf.rearrange("p t b -> p (t b)")
                    xb2 = xb.rearrange("p t b -> p (t b)")
                    ops = []
                    for g in range(4):
                        ops.append(lambda g=g: nc.tensor.matmul(
                            pall[:, g], bias_r[:, g], ones,
                            start=True, stop=True))
                        ops.append(lambda g=g: nc.tensor.matmul(
                            pall[:, g], w1_ih[:, 0, g], xf2,
                            start=False, stop=False, skip_group_check=True))
                        ops.append(lambda g=g: nc.tensor.matmul(
                            pall[:, g], w1_ih[:, 1, g], xb2,
                            start=False, stop=False, skip_group_check=True))
                    return ops

                xs = {0: load_x(0), 1: load_x(1)}
                pall = gpsum.tile([128, 4, NB], F32, tag="pall", name="p0")
                for op in bulk_ops_A(*xs[0], pall):
                    op()

                for c in range(NCH):
                    t0 = c * CH
                    pview = pall.rearrange("p g (t b) -> p g t b", t=CH)
                    if c + 2 < NCH:
                        xs[c + 2] = load_x(c + 2)
                    if c + 1 < NCH:
                        pall_n = gpsum.tile([128, 4, NB], F32, tag="pall",
                                            name=f"p{c + 1}")
                        pending = bulk_ops_A(*xs.pop(c + 1), pall_n)
                        xs.pop(c, None)
                    else:
                        pall_n, pending = None, []

                    for s in range(CH):
                        pos = t0 + s
                        if pos > 0:
                            for g in range(4):
                                nc.tensor.matmul(pview[:, g, s], w1_hh[:, g],
                                                 h1_sb[:, pos - 1],
                                                 start=False, stop=False,
                                                 skip_group_check=True)
                        # interleave next chunk's input-projection matmuls
                        q0, q1 = (s * 12) // CH, ((s + 1) * 12) // CH
                        for op in pending[q0:q1]:
                            op()

                        a_all = apool.tile([128, 4, BSH], BF16, tag="a_all")
                        nc.scalar.activation(a_all, pview[:, :, s], AF.Sigmoid)

                        u = apool.tile([128, BSH], BF16, tag="u")
                        nc.vector.scalar_tensor_tensor(
                            out=u, in0=a_all[:, 0], scalar=-0.5,
                            in1=a_all[:, 1], op0=ADD, op1=MUL)
                        v = apool.tile([128, BSH], F32, tag="v")
                        nc.gpsimd.tensor_tensor(out=v, in0=a_all[:, 2],
                                                in1=c_t, op=MUL)
                        c_n = spool.tile([128, BSH], F32, tag="c", name="c_n")
                        nc.vector.tensor_tensor(out=c_n, in0=u, in1=v, op=ADD)
                        tc_t = apool.tile([128, BSH], BF16, tag="tc_t")
                        nc.scalar.activation(tc_t, c_n, AF.Tanh, scale=2.0)
                        nc.vector.tensor_tensor(out=h1_sb[:, pos],
                                                in0=a_all[:, 3], in1=tc_t,
                                                op=MUL)
                        c_t = c_n
                    pall = pall_n

            # =============== PHASE B ===============
            with tc.tile_pool(name="hb", bufs=3) as hpool, \
                 tc.tile_pool(name="gb", bufs=2, space="PSUM") as gpsum2, \
                 tc.tile_pool(name="actb", bufs=3) as apool2, \
                 tc.tile_pool(name="stb", bufs=4) as spool2:

                h2_prev = spool2.tile([128, HB], BF16, tag="h2", name="h2_init")
                nc.vector.memset(h2_prev, 0.0)
                c2_t = spool2.tile([128, HB], F32, tag="c2", name="c2_init")
                nc.vector.memset(c2_t, 0.0)

                # h1_sb SBUF AP space: partition stride = free size (T*BSH),
                # free layout [T, BSH] with t-stride BSH.
                ppitch = T * BSH
                CHB = 16                # phase-B steps per PSUM tile
                NBB = CHB * HB          # 512: full bank per gate
                NCHB = T // CHB         # 64 chunks
                QB = CHB // 4           # copy slice size (timesteps)

                def ops_B(c, h1c, p2):
                    # copy slices first (gx matmuls read all of h1c), then
                    # bias + gx matmuls; consumed 20-ops-per-16-steps.
                    t0 = c * CHB
                    ops = []
                    for k in range(4):
                        ops.append(lambda k=k: nc.vector.tensor_copy(
                            h1c[0:64, k * QB:(k + 1) * QB],
                            h1_sb[0:64, t0 + k * QB:t0 + (k + 1) * QB]))
                        rev_in = bass.AP(
                            tensor=h1_sb.tensor,
                            offset=h1_sb.offset + 64 * ppitch
                            + (T - 1 - t0 - k * QB) * BSH,
                            ap=[[ppitch, 64], [-BSH, QB], [1, BSH]])
                        ops.append(lambda k=k, rev_in=rev_in:
                                   nc.vector.tensor_copy(
                                       h1c[64:128, k * QB:(k + 1) * QB],
                                       rev_in))
                    r0 = h1c[:, :, 0:HB]
                    r1 = h1c[:, :, HB:BSH]
                    for g in range(4):
                        ops.append(lambda g=g: nc.tensor.matmul(
                            p2[:, g], bias_r[:, 4 + g], ones,
                            start=True, stop=True))
                        ops.append(lambda g=g: nc.tensor.matmul(
                            p2[:, g], w2_ih[:, 0, g], r0,
                            start=False, stop=False, skip_group_check=True))
                        ops.append(lambda g=g: nc.tensor.matmul(
                            p2[:, g], w2_ih[:, 1, g], r1,
                            start=False, stop=False, skip_group_check=True))
                    return ops

                h1c = hpool.tile([128, CHB, BSH], BF16, tag="h1c", name="h0")
                p2 = gpsum2.tile([128, 4, NBB], F32, tag="p2", name="q0")
                for op in ops_B(0, h1c, p2):
                    op()

                for c in range(NCHB):
                    p2v = p2.rearrange("p g (t b) -> p g t b", t=CHB)
                    if c + 1 < NCHB:
                        h1c = hpool.tile([128, CHB, BSH], BF16, tag="h1c",
                                         name=f"h{c + 1}")
                        p2_n = gpsum2.tile([128, 4, NBB], F32, tag="p2",
                                           name=f"q{c + 1}")
                        pending = ops_B(c + 1, h1c, p2_n)
                    else:
                        p2_n, pending = None, []
                    nops = len(pending)

                    for s in range(CHB):
                        if c > 0 or s > 0:
                            for g in range(4):
                                nc.tensor.matmul(p2v[:, g, s], w2_hh[:, g],
                                                 h2_prev, start=False,
                                                 stop=False,
                                                 skip_group_check=True)
                        q0, q1 = (s * nops) // CHB, ((s + 1) * nops) // CHB
                        for op in pending[q0:q1]:
                            op()

                        a2 = apool2.tile([128, 4, HB], BF16, tag="a2")
                        nc.scalar.activation(a2, p2v[:, :, s], AF.Sigmoid)

                        u2 = apool2.tile([128, HB], BF16, tag="u2")
                        nc.vector.scalar_tensor_tensor(
                            out=u2, in0=a2[:, 0], scalar=-0.5,
                            in1=a2[:, 1], op0=ADD, op1=MUL)
                        v2 = apool2.tile([128, HB], F32, tag="v2")
                        nc.gpsimd.tensor_tensor(out=v2, in0=a2[:, 2],
                                                in1=c2_t, op=MUL)
                        c2_n = spool2.tile([128, HB], F32, tag="c2",
                                           name="c2_n")
                        nc.vector.tensor_tensor(out=c2_n, in0=u2, in1=v2,
                                                op=ADD)
                        tc2 = apool2.tile([128, HB], BF16, tag="tc2")
                        nc.scalar.activation(tc2, c2_n, AF.Tanh, scale=2.0)
                        h2_n = spool2.tile([128, HB], BF16, tag="h2",
                                           name="h2_n")
                        nc.vector.tensor_tensor(out=h2_n, in0=a2[:, 3],
                                                in1=tc2, op=MUL)
                        h2_prev = h2_n
                        c2_t = c2_n
                    p2 = p2_n

                # =============== PHASE C ===============
                h1l = apool2.tile([128, BSH], BF16)
                nc.vector.tensor_copy(h1l[0:64], h1_sb[0:64, T - 1])
                nc.vector.tensor_copy(h1l[64:128], h1_sb[64:128, 0])

                p3 = gpsum2.tile([128, 4, NB], F32, tag="p2")
                for g in range(4):
                    nc.tensor.matmul(p3[:, g, 0:HB], bias_r[:, 8 + g],
                                     ones[:, 0:HB], start=True, stop=True)
                    nc.tensor.matmul(p3[:, g, 0:HB], w2b_ih[:, 0, g],
                                     h1l[:, 0:HB], start=False, stop=False,
                                     skip_group_check=True)
                    nc.tensor.matmul(p3[:, g, 0:HB], w2b_ih[:, 1, g],
                                     h1l[:, HB:BSH], start=False,
                                     stop=False, skip_group_check=True)
                a3 = apool2.tile([128, 4, HB], BF16)
                nc.scalar.activation(a3, p3[:, :, 0:HB], AF.Sigmoid)
                u3 = apool2.tile([128, HB], F32)
                nc.vector.scalar_tensor_tensor(
                    out=u3, in0=a3[:, 0], scalar=-0.5, in1=a3[:, 1],
                    op0=ADD, op1=MUL)
                t3 = apool2.tile([128, HB], BF16)
                nc.scalar.activation(t3, u3, AF.Tanh, scale=2.0)
                h2b = apool2.tile([128, HB], BF16)
                nc.vector.tensor_tensor(out=h2b, in0=a3[:, 3], in1=t3, op=MUL)

                nc.sync.dma_start(out=h2cat[0:64, 0:HB], in_=h2_prev[0:64])
                nc.sync.dma_start(out=h2cat[0:64, HB:BSH], in_=h2_prev[64:128])
                nc.sync.dma_start(out=h2cat[64:128, 0:HB], in_=h2b[0:64])
                nc.sync.dma_start(out=h2cat[64:128, HB:BSH], in_=h2b[64:128])

                out_ps = gpsum2.tile([BSH, 1], F32, tag="p2")
                nc.tensor.matmul(out_ps, h2cat, fc_w, start=True, stop=True)
                out_sb = apool2.tile([BSH, 1], F32)
                nc.scalar.activation(out_sb, out_ps, AF.Identity, bias=fc_b)
                nc.sync.dma_start(out=out_d, in_=out_sb)

    nc.finalize()
    return nc


# PSUM gate-bank order [g, i, f, o]; PyTorch rows are [i, f, g, o].
GATE_SRC = [2, 0, 1, 3]
GATE_SCALE = [2.0, 1.0, 1.0, 1.0]  # g pre-scaled for the sigmoid/tanh trick


def _padih(wT_a, wT_b, K):
    # [K, 2, 4, 128]: stream a -> cols 0:64, stream b -> cols 64:128
    out = np.zeros((K, 2, 4, 128), np.float32)
    for k in range(4):
        gs, sc = GATE_SRC[k], GATE_SCALE[k]
        out[:, 0, k, 0:64] = sc * wT_a[:, gs * 64:(gs + 1) * 64]
        out[:, 1, k, 64:128] = sc * wT_b[:, gs * 64:(gs + 1) * 64]
    return out


def _blkdiag(wfT, wbT):
    out = np.zeros((128, 4, 128), np.float32)
    for k in range(4):
        gs, sc = GATE_SRC[k], GATE_SCALE[k]
        out[0:64, k, 0:64] = sc * wfT[:, gs * 64:(gs + 1) * 64]
        out[64:128, k, 64:128] = sc * wbT[:, gs * 64:(gs + 1) * 64]
    return out


def _prep_shared(w_ih, w_hh, b_ih, b_hh, fc_w, fc_b):
    b = (np.asarray(b_ih) + np.asarray(b_hh)).astype(np.float32)
    w_ih = np.asarray(w_ih, np.float32)
    w_hh = np.asarray(w_hh, np.float32)

    w1 = _padih(w_ih[0, 0].T, w_ih[0, 1].T, IN)
    w1h = _blkdiag(w_hh[0, 0].T, w_hh[0, 1].T)
    w2T = w_ih[1, 0].T
    w2 = _padih(w2T, w2T, 128)
    w2hT = w_hh[1, 0].T
    w2h = _blkdiag(w2hT, w2hT)
    w2bT = w_ih[1, 1].T
    w2b = _padih(w2bT, w2bT, 128)

    def bias_rows(bvec_f, bvec_b):
        out = np.zeros((4, 128), np.float32)
        for k in range(4):
            gs, sc = GATE_SRC[k], GATE_SCALE[k]
            out[k, 0:64] = sc * bvec_f[gs * 64:(gs + 1) * 64]
            out[k, 64:128] = sc * bvec_b[gs * 64:(gs + 1) * 64]
        return out

    br = np.zeros((1, 12, 128), np.float32)
    br[0, 0:4] = bias_rows(b[0, 0], b[0, 1])
    br[0, 4:8] = bias_rows(b[1, 0], b[1, 0])
    br[0, 8:12] = bias_rows(b[1, 1], b[1, 1])
    return {
        "w1_ih": np.ascontiguousarray(w1).astype(BF),
        "w1_hh": np.ascontiguousarray(w1h).astype(BF),
        "w2_ih": np.ascontiguousarray(w2).astype(BF),
        "w2_hh": np.ascontiguousarray(w2h).astype(BF),
        "w2b_ih": np.ascontiguousarray(w2b).astype(BF),
        "bias_rows": br.astype(BF),
        "fc_b": np.full((BSH, 1), float(np.asarray(fc_b).ravel()[0]), np.float32),
        "fc_w": np.ascontiguousarray(np.asarray(fc_w, np.float32).T).astype(BF),
    }


_NC_CACHE = {}


def _get_nc():
    key = "v2"
    if key not in _NC_CACHE:
        _NC_CACHE[key] = _build()
    return _NC_CACHE[key]


def _run(inputs, trace=False, tmpdir=None):
    x = np.asarray(inputs["x"], np.float32)
    shared = _prep_shared(inputs["w_ih"], inputs["w_hh"], inputs["b_ih"],
                          inputs["b_hh"], inputs["fc_w"], inputs["fc_b"])
    in_maps = []
    for c in range(N_CORES):
        xs = np.ascontiguousarray(
            x[c * BSH:(c + 1) * BSH].transpose(1, 2, 0)).astype(BF)  # [T, IN, BSH]
        m = dict(shared)
        m["x"] = xs
        in_maps.append(m)
    nc = _get_nc()
    res = run_bass_kernel_spmd(nc, in_maps, list(range(N_CORES)),
                               trace=trace, tmpdir=tmpdir)
    out = np.concatenate([res.results[c]["out"] for c in range(N_CORES)],
                         axis=0).astype(np.float32)
    return out, res


def kernel(x, w_ih, w_hh, b_ih, b_hh, fc_w, fc_b):
    out, _ = _run({"x": x, "w_ih": w_ih, "w_hh": w_hh, "b_ih": b_ih,
                   "b_hh": b_hh, "fc_w": fc_w, "fc_b": fc_b})
    return out


# revision 15
# speedup vs baseline: 1.8451x; 1.0426x over previous
"""BiLSTM (2-layer, H=64, T=1024, B=512) TRN2 Bass kernel.

Data-parallel over batch across 8 NeuronCores (B_shard=64/core); LSTM/FC
weights replicated. Per core, three phases:
  A: layer-1 fwd+bwd as one merged scan (PSUM banks = gate types in order
     [g,i,f,o], bank rows = [fwd-stream; bwd-stream]); input projections +
     biases enter PSUM via bulk matmuls (8 steps per bank), per-step
     recurrent matmuls (block-diag lhsT) accumulate on top.
  B: layer-2 fwd, same structure with bank rows = [batch 0:32; batch 32:64].
  C: layer-2 bwd needs only its t=T-1 step; FC head on device.

All matmul operands are bf16 (single-pass PE + fast weight load); PSUM and
the cell state stay fp32. Layer-1 hidden states live entirely in SBUF
(h1_sb, bf16) — no DRAM round-trip; the bwd stream's time reversal is
handled with negative-stride APs at consumption time.

Cell math per step (state c' = c/2, g-gate weights pre-scaled x2):
  sigmoid over [2g, i] and [f, o] banks -> a (bf16)
  u  = (sigma(2g) - 0.5) * i          (one scalar_tensor_tensor; = i*tanh(g)/2)
  v  = f * c'                          (tensor_tensor)
  c' = u + v                           (tensor_tensor)
  t  = Tanh(2*c')                      (activation with scale=2)
  h  = o * t  -> written into h1_sb[pos]
"""

import sys
import numpy as np

sys.path.insert(0, "/opt/trn_rl_repo")

import ml_dtypes  # noqa: E402

import concourse.bass as bass  # noqa: E402
import concourse.mybir as mybir  # noqa: E402
from concourse import bacc  # noqa: E402
from concourse.tile import TileContext  # noqa: E402
from concourse.bass_utils import run_bass_kernel_spmd  # noqa: E402

F32 = mybir.dt.float32
BF16 = mybir.dt.bfloat16
FP16 = mybir.dt.float16
AF = mybir.ActivationFunctionType
MUL = mybir.AluOpType.mult
ADD = mybir.AluOpType.add

T, IN, H, G = 1024, 128, 64, 256
B_FULL = 512
N_CORES = 8
BSH = B_FULL // N_CORES   # 64
CH = 8                    # timesteps per PSUM bank
NB = CH * BSH             # 512
HB = BSH // 2             # 32
NB2 = CH * HB             # 256
NCH = T // CH             # 128
BF = np.dtype(ml_dtypes.bfloat16)


def _build(num_devices=N_CORES):
    nc = bacc.Bacc("TRN2", target_bir_lowering=False, debug=False,
                   num_devices=num_devices)

    x_d = nc.dram_tensor("x", [T, IN, BSH], BF16, kind="ExternalInput").ap()
    w1_ih_d = nc.dram_tensor("w1_ih", [IN, 2, 4, 128], BF16, kind="ExternalInput").ap()
    w1_hh_d = nc.dram_tensor("w1_hh", [128, 4, 128], BF16, kind="ExternalInput").ap()
    w2_ih_d = nc.dram_tensor("w2_ih", [128, 2, 4, 128], BF16, kind="ExternalInput").ap()
    w2_hh_d = nc.dram_tensor("w2_hh", [128, 4, 128], BF16, kind="ExternalInput").ap()
    w2b_ih_d = nc.dram_tensor("w2b_ih", [128, 2, 4, 128], BF16, kind="ExternalInput").ap()
    bias_d = nc.dram_tensor("bias_rows", [1, 12, 128], BF16, kind="ExternalInput").ap()
    fcb_d = nc.dram_tensor("fc_b", [BSH, 1], F32, kind="ExternalInput").ap()
    fc_w_d = nc.dram_tensor("fc_w", [128, 1], BF16, kind="ExternalInput").ap()
    out_d = nc.dram_tensor("out", [BSH, 1], F32, kind="ExternalOutput").ap()

    def rev_x_ap(t_hi, ch):
        # x_d[t_hi - k, :, :] for k in 0..ch-1, laid out [IN, ch, BSH]
        tstr = IN * BSH
        return bass.AP(
            tensor=x_d.tensor,
            offset=x_d.offset + t_hi * tstr,
            ap=[[BSH, IN], [-tstr, ch], [1, BSH]])

    with TileContext(nc) as tc:
        with tc.tile_pool(name="singles", bufs=1) as singles:

            h1_sb = singles.tile([128, T, BSH], BF16)   # 128 KiB/partition

            w1_ih = singles.tile([IN, 2, 4, 128], BF16)
            w1_hh = singles.tile([128, 4, 128], BF16)
            w2_ih = singles.tile([128, 2, 4, 128], BF16)
            w2_hh = singles.tile([128, 4, 128], BF16)
            w2b_ih = singles.tile([128, 2, 4, 128], BF16)
            bias_r = singles.tile([1, 12, 128], BF16)
            ones = singles.tile([1, NB], BF16)
            fc_w = singles.tile([128, 1], BF16)
            fc_b = singles.tile([BSH, 1], F32)

            nc.sync.dma_start(out=w1_ih, in_=w1_ih_d)
            nc.sync.dma_start(out=w1_hh, in_=w1_hh_d)
            nc.sync.dma_start(out=w2_ih, in_=w2_ih_d)
            nc.sync.dma_start(out=w2_hh, in_=w2_hh_d)
            nc.sync.dma_start(out=w2b_ih, in_=w2b_ih_d)
            nc.sync.dma_start(out=bias_r, in_=bias_d)
            nc.sync.dma_start(out=fc_b, in_=fcb_d)
            nc.sync.dma_start(out=fc_w, in_=fc_w_d)
            nc.vector.memset(ones, 1.0)

            h2cat = singles.tile([128, BSH], BF16)

            # PE warmup: ~10us of dense matmuls flips the HAM clock gate to
            # 8/8 (2.4 GHz); steady-state PE gaps stay under the ~3.4us MID
            # window so it never re-throttles.
            warm_sb = singles.tile([128, NB], BF16)
            nc.vector.memset(warm_sb, 0.0)
            with tc.tile_pool(name="warm", bufs=1, space="PSUM") as wpsum:
                wp = wpsum.tile([128, NB], F32)
                for _ in range(24):
                    nc.tensor.matmul(wp, warm_sb[:, 0:128], warm_sb,
                                     start=True, stop=True)

            # =============== PHASE A ===============
            with tc.tile_pool(name="xa", bufs=4) as xpool, \
                 tc.tile_pool(name="ga", bufs=2, space="PSUM") as gpsum, \
                 tc.tile_pool(name="acta", bufs=3) as apool, \
                 tc.tile_pool(name="sta", bufs=4) as spool:

                c_t = spool.tile([128, BSH], FP16, tag="c", name="c_init")
                nc.vector.memset(c_t, 0.0)

                def load_x(c):
                    t0 = c * CH
                    xf = xpool.tile([IN, CH, BSH], BF16, tag="xf")
                    xb = xpool.tile([IN, CH, BSH], BF16, tag="xb")
                    nc.sync.dma_start(
                        out=xf, in_=x_d[t0:t0 + CH].rearrange("t p b -> p t b"))
                    nc.sync.dma_start(out=xb, in_=rev_x_ap(T - 1 - t0, CH))
                    return xf, xb

                def bulk_ops_A(xf, xb, pall):
                    xf2 = xf.rearrange("p t b -> p (t b)")
                    xb2 = xb.rearrange("p t b -> p (t b)")
                    ops = []
                    for g in range(4):
                        ops.append(lambda g=g: nc.tensor.matmul(
                            pall[:, g], bias_r[:, g], ones,
                            start=True, stop=True))
                        ops.append(lambda g=g: nc.tensor.matmul(
                            pall[:, g], w1_ih[:, 0, g], xf2,
                            start=False, stop=False, skip_group_check=True))
                        ops.append(lambda g=g: nc.tensor.matmul(
                            pall[:, g], w1_ih[:, 1, g], xb2,
                            start=False, stop=False, skip_group_check=True))
                    return ops

                xs = {0: load_x(0), 1: load_x(1)}
                pall = gpsum.tile([128, 4, NB], F32, tag="pall", name="p0")
                for op in bulk_ops_A(*xs[0], pall):
                    op()

                for c in range(NCH):
                    t0 = c * CH
                    pview = pall.rearrange("p g (t b) -> p g t b", t=CH)
                    if c + 2 < NCH:
                        xs[c + 2] = load_x(c + 2)
                    if c + 1 < NCH:
                        pall_n = gpsum.tile([128, 4, NB], F32, tag="pall",
                                            name=f"p{c + 1}")
                        pending = bulk_ops_A(*xs.pop(c + 1), pall_n)
                        xs.pop(c, None)
                    else:
                        pall_n, pending = None, []

                    for s in range(CH):
                        pos = t0 + s
                        if pos > 0:
                            for g in range(4):
                                nc.tensor.matmul(pview[:, g, s], w1_hh[:, g],
                                                 h1_sb[:, pos - 1],
                                                 start=False, stop=False,
                                                 skip_group_check=True)
                        # interleave next chunk's input-projection matmuls;
                        # start at step 2 (the psum tile frees around step 0)
                        if s >= 2:
                            q0 = ((s - 2) * 12) // (CH - 2)
                            q1 = ((s - 1) * 12) // (CH - 2)
                            for op in pending[q0:q1]:
                                op()

                        a_all = apool.tile([128, 4, BSH], BF16, tag="a_all")
                        nc.scalar.activation(a_all, pview[:, :, s], AF.Sigmoid)

                        u = apool.tile([128, BSH], FP16, tag="u")
                        nc.vector.scalar_tensor_tensor(
                            out=u, in0=a_all[:, 0], scalar=-0.5,
                            in1=a_all[:, 1], op0=ADD, op1=MUL)
                        v = apool.tile([128, BSH], FP16, tag="v")
                        nc.vector.tensor_tensor(out=v, in0=a_all[:, 2],
                                                in1=c_t, op=MUL)
                        c_n = spool.tile([128, BSH], FP16, tag="c", name="c_n")
                        nc.vector.tensor_tensor(out=c_n, in0=u, in1=v, op=ADD)
                        tc_t = apool.tile([128, BSH], BF16, tag="tc_t")
                        nc.scalar.activation(tc_t, c_n, AF.Tanh, scale=2.0)
                        nc.vector.tensor_tensor(out=h1_sb[:, pos],
                                                in0=a_all[:, 3], in1=tc_t,
                                                op=MUL)
                        c_t = c_n
                    pall = pall_n

            # =============== PHASE B ===============
            with tc.tile_pool(name="hb", bufs=3) as hpool, \
                 tc.tile_pool(name="gb", bufs=2, space="PSUM") as gpsum2, \
                 tc.tile_pool(name="actb", bufs=3) as apool2, \
                 tc.tile_pool(name="stb", bufs=4) as spool2:

                h2_prev = spool2.tile([128, HB], BF16, tag="h2", name="h2_init")
                nc.vector.memset(h2_prev, 0.0)
                c2_t = spool2.tile([128, HB], FP16, tag="c2", name="c2_init")
                nc.vector.memset(c2_t, 0.0)

                # h1_sb SBUF AP space: partition stride = free size (T*BSH),
                # free layout [T, BSH] with t-stride BSH.
                ppitch = T * BSH
                CHB = 16                # phase-B steps per PSUM tile
                NBB = CHB * HB          # 512: full bank per gate
                NCHB = T // CHB         # 64 chunks
                QB = CHB // 4           # copy slice size (timesteps)

                def ops_B(c, h1c, p2):
                    # copy slices first (gx matmuls read all of h1c), then
                    # bias + gx matmuls; consumed 20-ops-per-16-steps.
                    t0 = c * CHB
                    ops = []
                    for k in range(4):
                        ops.append(lambda k=k: nc.vector.tensor_copy(
                            h1c[0:64, k * QB:(k + 1) * QB],
                            h1_sb[0:64, t0 + k * QB:t0 + (k + 1) * QB]))
                        rev_in = bass.AP(
                            tensor=h1_sb.tensor,
                            offset=h1_sb.offset + 64 * ppitch
                            + (T - 1 - t0 - k * QB) * BSH,
                            ap=[[ppitch, 64], [-BSH, QB], [1, BSH]])
                        ops.append(lambda k=k, rev_in=rev_in:
                                   nc.vector.tensor_copy(
                                       h1c[64:128, k * QB:(k + 1) * QB],
                                       rev_in))
                    r0 = h1c[:, :, 0:HB]
                    r1 = h1c[:, :, HB:BSH]
                    for g in range(4):
                        ops.append(lambda g=g: nc.tensor.matmul(
                            p2[:, g], bias_r[:, 4 + g], ones,
                            start=True, stop=True))
                        ops.append(lambda g=g: nc.tensor.matmul(
                            p2[:, g], w2_ih[:, 0, g], r0,
                            start=False, stop=False, skip_group_check=True))
                        ops.append(lambda g=g: nc.tensor.matmul(
                            p2[:, g], w2_ih[:, 1, g], r1,
                            start=False, stop=False, skip_group_check=True))
                    return ops

                h1c = hpool.tile([128, CHB, BSH], BF16, tag="h1c", name="h0")
                p2 = gpsum2.tile([128, 4, NBB], F32, tag="p2", name="q0")
                for op in ops_B(0, h1c, p2):
                    op()

                for c in range(NCHB):
                    p2v = p2.rearrange("p g (t b) -> p g t b", t=CHB)
                    if c + 1 < NCHB:
                        h1c = hpool.tile([128, CHB, BSH], BF16, tag="h1c",
                                         name=f"h{c + 1}")
                        p2_n = gpsum2.tile([128, 4, NBB], F32, tag="p2",
                                           name=f"q{c + 1}")
                        pending = ops_B(c + 1, h1c, p2_n)
                    else:
                        p2_n, pending = None, []
                    nops = len(pending)

                    for s in range(CHB):
                        if c > 0 or s > 0:
                            for g in range(4):
                                nc.tensor.matmul(p2v[:, g, s], w2_hh[:, g],
                                                 h2_prev, start=False,
                                                 stop=False,
                                                 skip_group_check=True)
                        q0, q1 = (s * nops) // CHB, ((s + 1) * nops) // CHB
                        for op in pending[q0:q1]:
                            op()

                        a2 = apool2.tile([128, 4, HB], BF16, tag="a2")
                        nc.scalar.activation(a2, p2v[:, :, s], AF.Sigmoid)

                        u2 = apool2.tile([128, HB], FP16, tag="u2")
                        nc.vector.scalar_tensor_tensor(
                            out=u2, in0=a2[:, 0], scalar=-0.5,
                            in1=a2[:, 1], op0=ADD, op1=MUL)
                        v2 = apool2.tile([128, HB], FP16, tag="v2")
                        nc.vector.tensor_tensor(out=v2, in0=a2[:, 2],
                                                in1=c2_t, op=MUL)
                        c2_n = spool2.tile([128, HB], FP16, tag="c2",
                                           name="c2_n")
                        nc.vector.tensor_tensor(out=c2_n, in0=u2, in1=v2,
                                                op=ADD)
                        tc2 = apool2.tile([128, HB], BF16, tag="tc2")
                        nc.scalar.activation(tc2, c2_n, AF.Tanh, scale=2.0)
                        h2_n = spool2.tile([128, HB], BF16, tag="h2",
                                           name="h2_n")
                        nc.vector.tensor_tensor(out=h2_n, in0=a2[:, 3],
                                                in1=tc2, op=MUL)
                        h2_prev = h2_n
                        c2_t = c2_n
                    p2 = p2_n

                # =============== PHASE C ===============
                h1l = apool2.tile([128, BSH], BF16)
                nc.vector.tensor_copy(h1l[0:64], h1_sb[0:64, T - 1])
                nc.vector.tensor_copy(h1l[64:128], h1_sb[64:128, 0])

                p3 = gpsum2.tile([128, 4, NB], F32, tag="p2")
                for g in range(4):
                    nc.tensor.matmul(p3[:, g, 0:HB], bias_r[:, 8 + g],
                                     ones[:, 0:HB], start=True, stop=True)
                    nc.tensor.matmul(p3[:, g, 0:HB], w2b_ih[:, 0, g],
                                     h1l[:, 0:HB], start=False, stop=False,
                                     skip_group_check=True)
                    nc.tensor.matmul(p3[:, g, 0:HB], w2b_ih[:, 1, g],
                                     h1l[:, HB:BSH], start=False,
                                     stop=False, skip_group_check=True)
                a3 = apool2.tile([128, 4, HB], BF16)
                nc.scalar.activation(a3, p3[:, :, 0:HB], AF.Sigmoid)
                u3 = apool2.tile([128, HB], F32)
                nc.vector.scalar_tensor_tensor(
                    out=u3, in0=a3[:, 0], scalar=-0.5, in1=a3[:, 1],
                    op0=ADD, op1=MUL)
                t3 = apool2.tile([128, HB], BF16)
                nc.scalar.activation(t3, u3, AF.Tanh, scale=2.0)
                h2b = apool2.tile([128, HB], BF16)
                nc.vector.tensor_tensor(out=h2b, in0=a3[:, 3], in1=t3, op=MUL)

                nc.sync.dma_start(out=h2cat[0:64, 0:HB], in_=h2_prev[0:64])
                nc.sync.dma_start(out=h2cat[0:64, HB:BSH], in_=h2_prev[64:128])
                nc.sync.dma_start(out=h2cat[64:128, 0:HB], in_=h2b[0:64])
                nc.sync.dma_start(out=h2cat[64:128, HB:BSH], in_=h2b[64:128])

                out_ps = gpsum2.tile([BSH, 1], F32, tag="p2")
                nc.tensor.matmul(out_ps, h2cat, fc_w, start=True, stop=True)
                out_sb = apool2.tile([BSH, 1], F32)
                nc.scalar.activation(out_sb, out_ps, AF.Identity, bias=fc_b)
                nc.sync.dma_start(out=out_d, in_=out_sb)

    nc.finalize()
    return nc


# PSUM gate-bank order [g, i, f, o]; PyTorch rows are [i, f, g, o].
GATE_SRC = [2, 0, 1, 3]
GATE_SCALE = [2.0, 1.0, 1.0, 1.0]  # g pre-scaled for the sigmoid/tanh trick


def _padih(wT_a, wT_b, K):
    # [K, 2, 4, 128]: stream a -> cols 0:64, stream b -> cols 64:128
    out = np.zeros((K, 2, 4, 128), np.float32)
    for k in range(4):
        gs, sc = GATE_SRC[k], GATE_SCALE[k]
        out[:, 0, k, 0:64] = sc * wT_a[:, gs * 64:(gs + 1) * 64]
        out[:, 1, k, 64:128] = sc * wT_b[:, gs * 64:(gs + 1) * 64]
    return out


def _blkdiag(wfT, wbT):
    out = np.zeros((128, 4, 128), np.float32)
    for k in range(4):
        gs, sc = GATE_SRC[k], GATE_SCALE[k]
        out[0:64, k, 0:64] = sc * wfT[:, gs * 64:(gs + 1) * 64]
        out[64:128, k, 64:128] = sc * wbT[:, gs * 64:(gs + 1) * 64]
    return out


def _prep_shared(w_ih, w_hh, b_ih, b_hh, fc_w, fc_b):
    b = (np.asarray(b_ih) + np.asarray(b_hh)).astype(np.float32)
    w_ih = np.asarray(w_ih, np.float32)
    w_hh = np.asarray(w_hh, np.float32)

    w1 = _padih(w_ih[0, 0].T, w_ih[0, 1].T, IN)
    w1h = _blkdiag(w_hh[0, 0].T, w_hh[0, 1].T)
    w2T = w_ih[1, 0].T
    w2 = _padih(w2T, w2T, 128)
    w2hT = w_hh[1, 0].T
    w2h = _blkdiag(w2hT, w2hT)
    w2bT = w_ih[1, 1].T
    w2b = _padih(w2bT, w2bT, 128)

    def bias_rows(bvec_f, bvec_b):
        out = np.zeros((4, 128), np.float32)
        for k in range(4):
            gs, sc = GATE_SRC[k], GATE_SCALE[k]
            out[k, 0:64] = sc * bvec_f[gs * 64:(gs + 1) * 64]
            out[k, 64:128] = sc * bvec_b[gs * 64:(gs + 1) * 64]
        return out

    br = np.zeros((1, 12, 128), np.float32)
    br[0, 0:4] = bias_rows(b[0, 0], b[0, 1])
    br[0, 4:8] = bias_rows(b[1, 0], b[1, 0])
    br[0, 8:12] = bias_rows(b[1, 1], b[1, 1])
    return {
        "w1_ih": np.ascontiguousarray(w1).astype(BF),
        "w1_hh": np.ascontiguousarray(w1h).astype(BF),
        "w2_ih": np.ascontiguousarray(w2).astype(BF),
        "w2_hh": np.ascontiguousarray(w2h).astype(BF),
        "w2b_ih": np.ascontiguousarray(w2b).astype(BF),
        "bias_rows": br.astype(BF),
        "fc_b": np.full((BSH, 1), float(np.asarray(fc_b).ravel()[0]), np.float32),
        "fc_w": np.ascontiguousarray(np.asarray(fc_w, np.float32).T).astype(BF),
    }


_NC_CACHE = {}


def _get_nc():
    key = "v2"
    if key not in _NC_CACHE:
        _NC_CACHE[key] = _build()
    return _NC_CACHE[key]


def _run(inputs, trace=False, tmpdir=None):
    x = np.asarray(inputs["x"], np.float32)
    shared = _prep_shared(inputs["w_ih"], inputs["w_hh"], inputs["b_ih"],
                          inputs["b_hh"], inputs["fc_w"], inputs["fc_b"])
    in_maps = []
    for c in range(N_CORES):
        xs = np.ascontiguousarray(
            x[c * BSH:(c + 1) * BSH].transpose(1, 2, 0)).astype(BF)  # [T, IN, BSH]
        m = dict(shared)
        m["x"] = xs
        in_maps.append(m)
    nc = _get_nc()
    res = run_bass_kernel_spmd(nc, in_maps, list(range(N_CORES)),
                               trace=trace, tmpdir=tmpdir)
    out = np.concatenate([res.results[c]["out"] for c in range(N_CORES)],
                         axis=0).astype(np.float32)
    return out, res


def kernel(x, w_ih, w_hh, b_ih, b_hh, fc_w, fc_b):
    out, _ = _run({"x": x, "w_ih": w_ih, "w_hh": w_hh, "b_ih": b_ih,
                   "b_hh": b_hh, "fc_w": fc_w, "fc_b": fc_b})
    return out


# revision 18
# speedup vs baseline: 1.8454x; 1.0002x over previous
"""BiLSTM (2-layer, H=64, T=1024, B=512) TRN2 Bass kernel.

Data-parallel over batch across 8 NeuronCores (B_shard=64/core); LSTM/FC
weights replicated. Per core, three phases:
  A: layer-1 fwd+bwd as one merged scan (PSUM banks = gate types in order
     [g,i,f,o], bank rows = [fwd-stream; bwd-stream]); input projections +
     biases enter PSUM via bulk matmuls (8 steps per bank), per-step
     recurrent matmuls (block-diag lhsT) accumulate on top.
  B: layer-2 fwd, same structure with bank rows = [batch 0:32; batch 32:64].
  C: layer-2 bwd needs only its t=T-1 step; FC head on device.

All matmul operands are bf16 (single-pass PE + fast weight load); PSUM and
the cell state stay fp32. Layer-1 hidden states live entirely in SBUF
(h1_sb, bf16) — no DRAM round-trip; the bwd stream's time reversal is
handled with negative-stride APs at consumption time.

Cell math per step (state c' = c/2, g-gate weights pre-scaled x2):
  sigmoid over [2g, i] and [f, o] banks -> a (bf16)
  u  = (sigma(2g) - 0.5) * i          (one scalar_tensor_tensor; = i*tanh(g)/2)
  v  = f * c'                          (tensor_tensor)
  c' = u + v                           (tensor_tensor)
  t  = Tanh(2*c')                      (activation with scale=2)
  h  = o * t  -> written into h1_sb[pos]
"""

import sys
import numpy as np

sys.path.insert(0, "/opt/trn_rl_repo")

import ml_dtypes  # noqa: E402

import concourse.bass as bass  # noqa: E402
import concourse.mybir as mybir  # noqa: E402
from concourse import bacc  # noqa: E402
from concourse.tile import TileContext  # noqa: E402
from concourse.bass_utils import run_bass_kernel_spmd  # noqa: E402

F32 = mybir.dt.float32
BF16 = mybir.dt.bfloat16
FP16 = mybir.dt.float16
AF = mybir.ActivationFunctionType
MUL = mybir.AluOpType.mult
ADD = mybir.AluOpType.add

T, IN, H, G = 1024, 128, 64, 256
B_FULL = 512
N_CORES = 8
BSH = B_FULL // N_CORES   # 64
CH = 8                    # timesteps per PSUM bank
NB = CH * BSH             # 512
HB = BSH // 2             # 32
NB2 = CH * HB             # 256
NCH = T // CH             # 128
BF = np.dtype(ml_dtypes.bfloat16)


def _build(num_devices=N_CORES):
    nc = bacc.Bacc("TRN2", target_bir_lowering=False, debug=False,
                   num_devices=num_devices)

    x_d = nc.dram_tensor("x", [T, IN, BSH], BF16, kind="ExternalInput").ap()
    w1_ih_d = nc.dram_tensor("w1_ih", [IN, 2, 4, 128], BF16, kind="ExternalInput").ap()
    w1_hh_d = nc.dram_tensor("w1_hh", [128, 4, 128], BF16, kind="ExternalInput").ap()
    w2_ih_d = nc.dram_tensor("w2_ih", [128, 2, 4, 128], BF16, kind="ExternalInput").ap()
    w2_hh_d = nc.dram_tensor("w2_hh", [128, 4, 128], BF16, kind="ExternalInput").ap()
    w2b_ih_d = nc.dram_tensor("w2b_ih", [128, 2, 4, 128], BF16, kind="ExternalInput").ap()
    bias_d = nc.dram_tensor("bias_rows", [1, 12, 128], BF16, kind="ExternalInput").ap()
    fcb_d = nc.dram_tensor("fc_b", [BSH, 1], F32, kind="ExternalInput").ap()
    fc_w_d = nc.dram_tensor("fc_w", [128, 1], BF16, kind="ExternalInput").ap()
    out_d = nc.dram_tensor("out", [BSH, 1], F32, kind="ExternalOutput").ap()

    def rev_x_ap(t_hi, ch):
        # x_d[t_hi - k, :, :] for k in 0..ch-1, laid out [IN, ch, BSH]
        tstr = IN * BSH
        return bass.AP(
            tensor=x_d.tensor,
            offset=x_d.offset + t_hi * tstr,
            ap=[[BSH, IN], [-tstr, ch], [1, BSH]])

    with TileContext(nc) as tc:
        with tc.tile_pool(name="singles", bufs=1) as singles:

            h1_sb = singles.tile([128, T, BSH], BF16)   # 128 KiB/partition

            w1_ih = singles.tile([IN, 2, 4, 128], BF16)
            w1_hh = singles.tile([128, 4, 128], BF16)
            w2_ih = singles.tile([128, 2, 4, 128], BF16)
            w2_hh = singles.tile([128, 4, 128], BF16)
            w2b_ih = singles.tile([128, 2, 4, 128], BF16)
            bias_r = singles.tile([1, 12, 128], BF16)
            ones = singles.tile([1, NB], BF16)
            fc_w = singles.tile([128, 1], BF16)
            fc_b = singles.tile([BSH, 1], F32)

            nc.sync.dma_start(out=w1_ih, in_=w1_ih_d)
            nc.sync.dma_start(out=w1_hh, in_=w1_hh_d)
            nc.sync.dma_start(out=w2_ih, in_=w2_ih_d)
            nc.sync.dma_start(out=w2_hh, in_=w2_hh_d)
            nc.sync.dma_start(out=w2b_ih, in_=w2b_ih_d)
            nc.sync.dma_start(out=bias_r, in_=bias_d)
            nc.sync.dma_start(out=fc_b, in_=fcb_d)
            nc.sync.dma_start(out=fc_w, in_=fc_w_d)
            nc.vector.memset(ones, 1.0)

            h2cat = singles.tile([128, BSH], BF16)

            # PE warmup: ~10us of dense matmuls flips the HAM clock gate to
            # 8/8 (2.4 GHz); steady-state PE gaps stay under the ~3.4us MID
            # window so it never re-throttles.
            warm_sb = singles.tile([128, NB], BF16)
            nc.vector.memset(warm_sb, 0.0)
            with tc.tile_pool(name="warm", bufs=1, space="PSUM") as wpsum:
                wp = wpsum.tile([128, NB], F32)
                for _ in range(24):
                    nc.tensor.matmul(wp, warm_sb[:, 0:128], warm_sb,
                                     start=True, stop=True)

            # =============== PHASE A ===============
            with tc.tile_pool(name="xa", bufs=4) as xpool, \
                 tc.tile_pool(name="ga", bufs=2, space="PSUM") as gpsum, \
                 tc.tile_pool(name="acta", bufs=3) as apool, \
                 tc.tile_pool(name="sta", bufs=4) as spool:

                c_t = spool.tile([128, BSH], FP16, tag="c", name="c_init")
                nc.vector.memset(c_t, 0.0)

                def load_x(c):
                    t0 = c * CH
                    xf = xpool.tile([IN, CH, BSH], BF16, tag="xf")
                    xb = xpool.tile([IN, CH, BSH], BF16, tag="xb")
                    nc.sync.dma_start(
                        out=xf, in_=x_d[t0:t0 + CH].rearrange("t p b -> p t b"))
                    nc.sync.dma_start(out=xb, in_=rev_x_ap(T - 1 - t0, CH))
                    return xf, xb

                def bulk_ops_A(xf, xb, pall):
                    xf2 = xf.rearrange("p t b -> p (t b)")
                    xb2 = xb.rearrange("p t b -> p (t b)")
                    ops = []
                    for g in range(4):
                        ops.append(lambda g=g: nc.tensor.matmul(
                            pall[:, g], bias_r[:, g], ones,
                            start=True, stop=True))
                        ops.append(lambda g=g: nc.tensor.matmul(
                            pall[:, g], w1_ih[:, 0, g], xf2,
                            start=False, stop=False, skip_group_check=True))
                        ops.append(lambda g=g: nc.tensor.matmul(
                            pall[:, g], w1_ih[:, 1, g], xb2,
                            start=False, stop=False, skip_group_check=True))
                    return ops

                xs = {0: load_x(0), 1: load_x(1)}
                pall = gpsum.tile([128, 4, NB], F32, tag="pall", name="p0")
                # re-warm the PE right after the first x DMA lands (the DMA
                # wait is the idle window that re-throttles it); these queue
                # ahead of the real chunk-0 matmuls which overwrite bank 0.
                xw = xs[0][0].rearrange("p t b -> p (t b)")
                for _ in range(10):
                    nc.tensor.matmul(pall[:, 0], w1_ih[:, 0, 0], xw,
                                     start=True, stop=True)
                for op in bulk_ops_A(*xs[0], pall):
                    op()

                for c in range(NCH):
                    t0 = c * CH
                    pview = pall.rearrange("p g (t b) -> p g t b", t=CH)
                    if c + 2 < NCH:
                        xs[c + 2] = load_x(c + 2)
                    if c + 1 < NCH:
                        pall_n = gpsum.tile([128, 4, NB], F32, tag="pall",
                                            name=f"p{c + 1}")
                        pending = bulk_ops_A(*xs.pop(c + 1), pall_n)
                        xs.pop(c, None)
                    else:
                        pall_n, pending = None, []

                    for s in range(CH):
                        pos = t0 + s
                        if pos > 0:
                            for g in range(4):
                                nc.tensor.matmul(pview[:, g, s], w1_hh[:, g],
                                                 h1_sb[:, pos - 1],
                                                 start=False, stop=False,
                                                 skip_group_check=True)
                        # interleave next chunk's input-projection matmuls;
                        # start at step 2 (the psum tile frees around step 0)
                        if s >= 2:
                            q0 = ((s - 2) * 12) // (CH - 2)
                            q1 = ((s - 1) * 12) // (CH - 2)
                            for op in pending[q0:q1]:
                                op()

                        a_all = apool.tile([128, 4, BSH], BF16, tag="a_all")
                        nc.scalar.activation(a_all, pview[:, :, s], AF.Sigmoid)

                        u = apool.tile([128, BSH], FP16, tag="u")
                        nc.vector.scalar_tensor_tensor(
                            out=u, in0=a_all[:, 0], scalar=-0.5,
                            in1=a_all[:, 1], op0=ADD, op1=MUL)
                        v = apool.tile([128, BSH], FP16, tag="v")
                        nc.vector.tensor_tensor(out=v, in0=a_all[:, 2],
                                                in1=c_t, op=MUL)
                        c_n = spool.tile([128, BSH], FP16, tag="c", name="c_n")
                        nc.vector.tensor_tensor(out=c_n, in0=u, in1=v, op=ADD)
                        tc_t = apool.tile([128, BSH], BF16, tag="tc_t")
                        nc.scalar.activation(tc_t, c_n, AF.Tanh, scale=2.0)
                        nc.vector.tensor_tensor(out=h1_sb[:, pos],
                                                in0=a_all[:, 3], in1=tc_t,
                                                op=MUL)
                        c_t = c_n
                    pall = pall_n

            # =============== PHASE B ===============
            with tc.tile_pool(name="hb", bufs=3) as hpool, \
                 tc.tile_pool(name="gb", bufs=2, space="PSUM") as gpsum2, \
                 tc.tile_pool(name="actb", bufs=3) as apool2, \
                 tc.tile_pool(name="stb", bufs=4) as spool2:

                h2_prev = spool2.tile([128, HB], BF16, tag="h2", name="h2_init")
                nc.vector.memset(h2_prev, 0.0)
                c2_t = spool2.tile([128, HB], FP16, tag="c2", name="c2_init")
                nc.vector.memset(c2_t, 0.0)

                # h1_sb SBUF AP space: partition stride = free size (T*BSH),
                # free layout [T, BSH] with t-stride BSH.
                ppitch = T * BSH
                CHB = 16                # phase-B steps per PSUM tile
                NBB = CHB * HB          # 512: full bank per gate
                NCHB = T // CHB         # 64 chunks
                QB = CHB // 4           # copy slice size (timesteps)

                def ops_B(c, h1c, p2):
                    # copy slices first (gx matmuls read all of h1c), then
                    # bias + gx matmuls; consumed 20-ops-per-16-steps.
                    t0 = c * CHB
                    ops = []
                    for k in range(4):
                        ops.append(lambda k=k: nc.vector.tensor_copy(
                            h1c[0:64, k * QB:(k + 1) * QB],
                            h1_sb[0:64, t0 + k * QB:t0 + (k + 1) * QB]))
                        rev_in = bass.AP(
                            tensor=h1_sb.tensor,
                            offset=h1_sb.offset + 64 * ppitch
                            + (T - 1 - t0 - k * QB) * BSH,
                            ap=[[ppitch, 64], [-BSH, QB], [1, BSH]])
                        ops.append(lambda k=k, rev_in=rev_in:
                                   nc.vector.tensor_copy(
                                       h1c[64:128, k * QB:(k + 1) * QB],
                                       rev_in))
                    r0 = h1c[:, :, 0:HB]
                    r1 = h1c[:, :, HB:BSH]
                    for g in range(4):
                        ops.append(lambda g=g: nc.tensor.matmul(
                            p2[:, g], bias_r[:, 4 + g], ones,
                            start=True, stop=True))
                        ops.append(lambda g=g: nc.tensor.matmul(
                            p2[:, g], w2_ih[:, 0, g], r0,
                            start=False, stop=False, skip_group_check=True))
                        ops.append(lambda g=g: nc.tensor.matmul(
                            p2[:, g], w2_ih[:, 1, g], r1,
                            start=False, stop=False, skip_group_check=True))
                    return ops

                h1c = hpool.tile([128, CHB, BSH], BF16, tag="h1c", name="h0")
                p2 = gpsum2.tile([128, 4, NBB], F32, tag="p2", name="q0")
                ops0 = ops_B(0, h1c, p2)
                for op in ops0[:8]:     # h1c copy slices
                    op()
                # re-warm the PE after the phase-A -> phase-B idle gap
                hw_ = h1c[:, 0:CHB // 2].rearrange("p t b -> p (t b)")
                for _ in range(10):
                    nc.tensor.matmul(p2[:, 0], w2_ih[:, 0, 0], hw_,
                                     start=True, stop=True)
                for op in ops0[8:]:     # bias + gx matmuls
                    op()

                for c in range(NCHB):
                    p2v = p2.rearrange("p g (t b) -> p g t b", t=CHB)
                    if c + 1 < NCHB:
                        h1c = hpool.tile([128, CHB, BSH], BF16, tag="h1c",
                                         name=f"h{c + 1}")
                        p2_n = gpsum2.tile([128, 4, NBB], F32, tag="p2",
                                           name=f"q{c + 1}")
                        pending = ops_B(c + 1, h1c, p2_n)
                    else:
                        p2_n, pending = None, []
                    nops = len(pending)

                    for s in range(CHB):
                        if c > 0 or s > 0:
                            for g in range(4):
                                nc.tensor.matmul(p2v[:, g, s], w2_hh[:, g],
                                                 h2_prev, start=False,
                                                 stop=False,
                                                 skip_group_check=True)
                        q0, q1 = (s * nops) // CHB, ((s + 1) * nops) // CHB
                        for op in pending[q0:q1]:
                            op()

                        a2 = apool2.tile([128, 4, HB], BF16, tag="a2")
                        nc.scalar.activation(a2, p2v[:, :, s], AF.Sigmoid)

                        u2 = apool2.tile([128, HB], FP16, tag="u2")
                        nc.vector.scalar_tensor_tensor(
                            out=u2, in0=a2[:, 0], scalar=-0.5,
                            in1=a2[:, 1], op0=ADD, op1=MUL)
                        v2 = apool2.tile([128, HB], FP16, tag="v2")
                        nc.vector.tensor_tensor(out=v2, in0=a2[:, 2],
                                                in1=c2_t, op=MUL)
                        c2_n = spool2.tile([128, HB], FP16, tag="c2",
                                           name="c2_n")
                        nc.vector.tensor_tensor(out=c2_n, in0=u2, in1=v2,
                                                op=ADD)
                        tc2 = apool2.tile([128, HB], BF16, tag="tc2")
                        nc.scalar.activation(tc2, c2_n, AF.Tanh, scale=2.0)
                        h2_n = spool2.tile([128, HB], BF16, tag="h2",
                                           name="h2_n")
                        nc.vector.tensor_tensor(out=h2_n, in0=a2[:, 3],
                                                in1=tc2, op=MUL)
                        h2_prev = h2_n
                        c2_t = c2_n
                    p2 = p2_n

                # =============== PHASE C ===============
                h1l = apool2.tile([128, BSH], BF16)
                nc.vector.tensor_copy(h1l[0:64], h1_sb[0:64, T - 1])
                nc.vector.tensor_copy(h1l[64:128], h1_sb[64:128, 0])

                p3 = gpsum2.tile([128, 4, NB], F32, tag="p2")
                for g in range(4):
                    nc.tensor.matmul(p3[:, g, 0:HB], bias_r[:, 8 + g],
                                     ones[:, 0:HB], start=True, stop=True)
                    nc.tensor.matmul(p3[:, g, 0:HB], w2b_ih[:, 0, g],
                                     h1l[:, 0:HB], start=False, stop=False,
                                     skip_group_check=True)
                    nc.tensor.matmul(p3[:, g, 0:HB], w2b_ih[:, 1, g],
                                     h1l[:, HB:BSH], start=False,
                                     stop=False, skip_group_check=True)
                a3 = apool2.tile([128, 4, HB], BF16)
                nc.scalar.activation(a3, p3[:, :, 0:HB], AF.Sigmoid)
                u3 = apool2.tile([128, HB], F32)
                nc.vector.scalar_tensor_tensor(
                    out=u3, in0=a3[:, 0], scalar=-0.5, in1=a3[:, 1],
                    op0=ADD, op1=MUL)
                t3 = apool2.tile([128, HB], BF16)
                nc.scalar.activation(t3, u3, AF.Tanh, scale=2.0)
                h2b = apool2.tile([128, HB], BF16)
                nc.vector.tensor_tensor(out=h2b, in0=a3[:, 3], in1=t3, op=MUL)

                nc.sync.dma_start(out=h2cat[0:64, 0:HB], in_=h2_prev[0:64])
                nc.sync.dma_start(out=h2cat[0:64, HB:BSH], in_=h2_prev[64:128])
                nc.sync.dma_start(out=h2cat[64:128, 0:HB], in_=h2b[0:64])
                nc.sync.dma_start(out=h2cat[64:128, HB:BSH], in_=h2b[64:128])

                out_ps = gpsum2.tile([BSH, 1], F32, tag="p2")
                nc.tensor.matmul(out_ps, h2cat, fc_w, start=True, stop=True)
                out_sb = apool2.tile([BSH, 1], F32)
                nc.scalar.activation(out_sb, out_ps, AF.Identity, bias=fc_b)
                nc.sync.dma_start(out=out_d, in_=out_sb)

    nc.finalize()
    return nc


# PSUM gate-bank order [g, i, f, o]; PyTorch rows are [i, f, g, o].
GATE_SRC = [2, 0, 1, 3]
GATE_SCALE = [2.0, 1.0, 1.0, 1.0]  # g pre-scaled for the sigmoid/tanh trick


def _padih(wT_a, wT_b, K):
    # [K, 2, 4, 128]: stream a -> cols 0:64, stream b -> cols 64:128
    out = np.zeros((K, 2, 4, 128), np.float32)
    for k in range(4):
        gs, sc = GATE_SRC[k], GATE_SCALE[k]
        out[:, 0, k, 0:64] = sc * wT_a[:, gs * 64:(gs + 1) * 64]
        out[:, 1, k, 64:128] = sc * wT_b[:, gs * 64:(gs + 1) * 64]
    return out


def _blkdiag(wfT, wbT):
    out = np.zeros((128, 4, 128), np.float32)
    for k in range(4):
        gs, sc = GATE_SRC[k], GATE_SCALE[k]
        out[0:64, k, 0:64] = sc * wfT[:, gs * 64:(gs + 1) * 64]
        out[64:128, k, 64:128] = sc * wbT[:, gs * 64:(gs + 1) * 64]
    return out


def _prep_shared(w_ih, w_hh, b_ih, b_hh, fc_w, fc_b):
    b = (np.asarray(b_ih) + np.asarray(b_hh)).astype(np.float32)
    w_ih = np.asarray(w_ih, np.float32)
    w_hh = np.asarray(w_hh, np.float32)

    w1 = _padih(w_ih[0, 0].T, w_ih[0, 1].T, IN)
    w1h = _blkdiag(w_hh[0, 0].T, w_hh[0, 1].T)
    w2T = w_ih[1, 0].T
    w2 = _padih(w2T, w2T, 128)
    w2hT = w_hh[1, 0].T
    w2h = _blkdiag(w2hT, w2hT)
    w2bT = w_ih[1, 1].T
    w2b = _padih(w2bT, w2bT, 128)

    def bias_rows(bvec_f, bvec_b):
        out = np.zeros((4, 128), np.float32)
        for k in range(4):
            gs, sc = GATE_SRC[k], GATE_SCALE[k]
            out[k, 0:64] = sc * bvec_f[gs * 64:(gs + 1) * 64]
            out[k, 64:128] = sc * bvec_b[gs * 64:(gs + 1) * 64]
        return out

    br = np.zeros((1, 12, 128), np.float32)
    br[0, 0:4] = bias_rows(b[0, 0], b[0, 1])
    br[0, 4:8] = bias_rows(b[1, 0], b[1, 0])
    br[0, 8:12] = bias_rows(b[1, 1], b[1, 1])
    return {
        "w1_ih": np.ascontiguousarray(w1).astype(BF),
        "w1_hh": np.ascontiguousarray(w1h).astype(BF),
        "w2_ih": np.ascontiguousarray(w2).astype(BF),
        "w2_hh": np.ascontiguousarray(w2h).astype(BF),
        "w2b_ih": np.ascontiguousarray(w2b).astype(BF),
        "bias_rows": br.astype(BF),
        "fc_b": np.full((BSH, 1), float(np.asarray(fc_b).ravel()[0]), np.float32),
        "fc_w": np.ascontiguousarray(np.asarray(fc_w, np.float32).T).astype(BF),
    }


_NC_CACHE = {}


def _get_nc():
    key = "v2"
    if key not in _NC_CACHE:
        _NC_CACHE[key] = _build()
    return _NC_CACHE[key]


def _run(inputs, trace=False, tmpdir=None):
    x = np.asarray(inputs["x"], np.float32)
    shared = _prep_shared(inputs["w_ih"], inputs["w_hh"], inputs["b_ih"],
                          inputs["b_hh"], inputs["fc_w"], inputs["fc_b"])
    in_maps = []
    for c in range(N_CORES):
        xs = np.ascontiguousarray(
            x[c * BSH:(c + 1) * BSH].transpose(1, 2, 0)).astype(BF)  # [T, IN, BSH]
        m = dict(shared)
        m["x"] = xs
        in_maps.append(m)
    nc = _get_nc()
    res = run_bass_kernel_spmd(nc, in_maps, list(range(N_CORES)),
                               trace=trace, tmpdir=tmpdir)
    out = np.concatenate([res.results[c]["out"] for c in range(N_CORES)],
                         axis=0).astype(np.float32)
    return out, res


def kernel(x, w_ih, w_hh, b_ih, b_hh, fc_w, fc_b):
    out, _ = _run({"x": x, "w_ih": w_ih, "w_hh": w_hh, "b_ih": b_ih,
                   "b_hh": b_hh, "fc_w": fc_w, "fc_b": fc_b})
    return out
